# revision 30
# baseline (speedup 1.0000x reference)
# Trainium2 Bass kernel for nn_BatchelorAdj (motion-compensated MRI recon
# adjoint):  out = sum_t W_t^T( sum_c conj(S_c) . IFFT2c(K_c . M_ct) )
#
# v4 design (cost-model driven):
#  - host precomputes km = kspace*mask per (coil,frame) in a stacked-chunk
#    bf16 layout so each IFFT matmul pass runs 5-instruction chains over a
#    640-long stacked contraction ([kmr;kmi] x [A_r;-A_i]); pass 1 merges
#    the 64-row x-tail of the r/i outputs into one 128-row chain, and pass
#    2 merges the two comps' x-tail chains via a stacked ASPM constant
#    (psum rows 0:64 = re tail, 64:128 = im tail) + partition-shift DMA.
#  - smaps: 2 planes (sr, si) resident in SBUF for all 16 coils (+2 f24
#    slots); the second coil-combine product pair uses a negative-stride
#    plane view of the image tile, so no duplicated sr plane is shipped.
#  - adjoint warp: host ships bf16 bilinear hat tables with per-(slot,bx)
#    widths and per-(slot,bx,ti) start offsets that are the union of the 8
#    cores' local windows (shared program => shared offsets), packed flat
#    in DRAM for full-rate DMA.
#  - schedule is slot-major: slot0 jobs; slot1 jobs || warp0; f24 jobs +
#    slot2 jobs || warp1+warp3; tail warp2. Warp table DMA spreads over
#    the whole fft timeline.
#  - per-bx psum bands [128, 324+2D] are flushed into per-bx bf16 SBUF
#    accumulators (Act copy on the first slot, DVE add after); host
#    applies x-shifts in the final cross-core reduce.
import math
import numpy as np

Nx = Ny = 320
Nc = 16
Nt = 25
NCORES = 8
BX, BY = 32, 4
NTX, NTY = Nx // BX, Ny // BY       # 10, 80
FR_FULL = 3
C24 = Nc // NCORES                  # 2
NSLOT = FR_FULL + 1                 # 4
NJOB = FR_FULL * Nc + C24           # 50 fft jobs per core
KMW = 2176                          # km tile width (see _km_pack)

_CACHE = {}


def _build_A():
    j = np.arange(Nx)
    F = np.exp(2j * np.pi * np.outer(j, j) / Nx) / np.sqrt(Nx)
    P = np.zeros((Nx, Nx))
    P[j, (j + Nx // 2) % Nx] = 1.0
    A = P @ F @ P
    return A.real.astype(np.float64), A.imag.astype(np.float64)


def _stack5(M, N):
    """Chunk the 640-row stack [M;N] of two 320-row mats into [5,128,cols]:
    (M0, N0, M1, N1, [M2;N2])."""
    out = np.zeros((5, 128, M.shape[1]), np.float64)
    out[0] = M[0:128]
    out[1] = N[0:128]
    out[2] = M[128:256]
    out[3] = N[128:256]
    out[4, 0:64] = M[256:320]
    out[4, 64:128] = N[256:320]
    return out


def _km_pack(kmr, kmi):
    """km [y,x] pair -> [128, 2176] layout.
    cols [0:1600]:   km5 chunks (kmr0,kmi0,kmr1,kmi1,[kmr2;kmi2]) full x.
    cols [1600:1664]: [kmi2 ; -kmr2] @ x 256:320 (partition-stacked)
    cols [1664:1920]: (kmi-y0, -kmr-y0, kmi-y1, -kmr-y1) @ x 256:320
    cols [1920:2176]: (kmr-y0, kmi-y0, kmr-y1, kmi-y1) @ x 256:320."""
    out = np.zeros((128, KMW), np.float64)
    km5 = _stack5(kmr, kmi)
    out[:, 0:1600] = km5.transpose(1, 0, 2).reshape(128, 1600)
    out[0:64, 1600:1664] = kmi[256:320, 256:320]
    out[64:128, 1600:1664] = -kmr[256:320, 256:320]
    out[:, 1664:1728] = kmi[0:128, 256:320]
    out[:, 1728:1792] = -kmr[0:128, 256:320]
    out[:, 1792:1856] = kmi[128:256, 256:320]
    out[:, 1856:1920] = -kmr[128:256, 256:320]
    out[:, 1920:1984] = kmr[0:128, 256:320]
    out[:, 1984:2048] = kmi[0:128, 256:320]
    out[:, 2048:2112] = kmr[128:256, 256:320]
    out[:, 2112:2176] = kmi[128:256, 256:320]
    return out


def _geometry(flow):
    """Union-local warp windows shared by all 8 cores.

    Returns dict with per-(slot,bx,ti) y/x window starts (sy, sx), per-
    (slot,bx) widths (JLb, WLb), flat ex/ey table offsets, and Dx/Dy."""
    f64 = np.float64
    Dx = int(math.ceil(np.abs(flow[:, :, 0, :]).max()))
    Dy = int(math.ceil(np.abs(flow[:, :, 1, :]).max()))
    W = 32 + 2 * Dx
    Xg, Yg = np.meshgrid(np.arange(Nx, dtype=f64), np.arange(Ny, dtype=f64),
                         indexing="ij")
    fminx = np.zeros((Nt, NTX, NTY), np.int64)
    fmaxx = np.zeros((Nt, NTX, NTY), np.int64)
    fminy = np.zeros((Nt, NTX, NTY), np.int64)
    fmaxy = np.zeros((Nt, NTX, NTY), np.int64)
    for t in range(Nt):
        px = np.clip(Xg + flow[:, :, 0, t].astype(f64), 0.0, Nx - 1.0)
        py = np.clip(Yg + flow[:, :, 1, t].astype(f64), 0.0, Ny - 1.0)
        fx = np.floor(px).astype(np.int64).reshape(NTX, BX, NTY, BY)
        fy = np.floor(py).astype(np.int64).reshape(NTX, BX, NTY, BY)
        fminx[t] = fx.min(axis=(1, 3))
        fmaxx[t] = fx.max(axis=(1, 3))
        fminy[t] = fy.min(axis=(1, 3))
        fmaxy[t] = fy.max(axis=(1, 3))

    sx = np.zeros((NSLOT, NTX, NTY), np.int64)
    ex_hi = np.zeros((NSLOT, NTX, NTY), np.int64)
    sy = np.zeros((NSLOT, NTX, NTY), np.int64)
    ey_hi = np.zeros((NSLOT, NTX, NTY), np.int64)
    for s in range(FR_FULL):
        frames = [FR_FULL * r + s for r in range(NCORES)]
        sx[s] = fminx[frames].min(axis=0)
        ex_hi[s] = fmaxx[frames].max(axis=0)
        sy[s] = fminy[frames].min(axis=0)
        ey_hi[s] = fmaxy[frames].max(axis=0)
    sx[FR_FULL], ex_hi[FR_FULL] = fminx[Nt - 1], fmaxx[Nt - 1]
    sy[FR_FULL], ey_hi[FR_FULL] = fminy[Nt - 1], fmaxy[Nt - 1]

    # x windows: PE psum writes need base partition in {0,32,64}, so pin
    # the window start at the band start and trim only the upper edge.
    bxlo = (32 * np.arange(NTX) - Dx)[None, :, None]
    sx = np.broadcast_to(bxlo, sx.shape).copy()
    WLb = np.minimum((ex_hi - sx + 2).max(axis=2), W)   # [NSLOT, NTX]
    JLb = (ey_hi - sy + 2).max(axis=2)
    sy = np.minimum(sy, Ny + Dy + 4 - JLb[:, :, None])

    ex_off = np.zeros((NSLOT, NTX), np.int64)
    ey_off = np.zeros((NSLOT, NTX), np.int64)
    pos = 0
    for s in range(NSLOT):
        for bx in range(NTX):
            ex_off[s, bx] = pos
            pos += 128 * NTY * int(WLb[s, bx])
    ex_tot = pos
    pos = 0
    for s in range(NSLOT):
        for bx in range(NTX):
            ey_off[s, bx] = pos
            pos += 128 * int(JLb[s, bx]) * NTY
    ey_tot = pos
    return dict(Dx=Dx, Dy=Dy, W=W, sx=sx, sy=sy, WLb=WLb, JLb=JLb,
                ex_off=ex_off, ey_off=ey_off, ex_tot=ex_tot, ey_tot=ey_tot)


def _build_program(g):
    from concourse import bass, bacc, tile, mybir

    Dx, Dy, W = g["Dx"], g["Dy"], g["W"]
    PWW = Ny + 2 * Dy + 4              # psum band columns (y + Dy offset)
    oy = g["sy"] + Dy                  # psum band col starts
    ox = g["sx"] - (32 * np.arange(NTX) - Dx)[None, :, None]  # band rows
    WLb, JLb = g["WLb"], g["JLb"]
    f32 = mybir.dt.float32
    bf16 = mybir.dt.bfloat16
    MULT = mybir.AluOpType.mult
    ADD = mybir.AluOpType.add
    SUB = mybir.AluOpType.subtract
    ACTF = mybir.ActivationFunctionType

    nc = bacc.Bacc("TRN2", target_bir_lowering=False, debug=False,
                   num_devices=NCORES)

    kmX = nc.dram_tensor("kmX", [NJOB, 128, KMW], bf16, kind="ExternalInput")
    smX = nc.dram_tensor("smX", [Nc + C24, 128, 2, 3, Ny], bf16,
                         kind="ExternalInput")
    ASd = nc.dram_tensor("ASd", [2, 5, 128, Ny], bf16, kind="ExternalInput")
    ASPMd = nc.dram_tensor("ASPMd", [5, 128, 128], bf16,
                           kind="ExternalInput")
    exd = nc.dram_tensor("exd", [g["ex_tot"]], bf16, kind="ExternalInput")
    eyd = nc.dram_tensor("eyd", [g["ey_tot"]], bf16, kind="ExternalInput")
    zzd = nc.dram_tensor("zzd", [1, 512], bf16, kind="ExternalInput")
    outp = nc.dram_tensor("outp", [NTX, 2, 128, Ny], bf16,
                          kind="ExternalOutput")

    from contextlib import ExitStack
    with tile.TileContext(nc) as tc, ExitStack() as ctx:
        const_pool = ctx.enter_context(tc.tile_pool(name="const", bufs=1))
        acc_pool = ctx.enter_context(tc.tile_pool(name="acc", bufs=1))
        aux_pool = ctx.enter_context(tc.tile_pool(name="aux", bufs=1))
        sm_pool = ctx.enter_context(tc.tile_pool(name="sm", bufs=1))
        sm24_pool = ctx.enter_context(tc.tile_pool(name="sm24", bufs=2))
        km_pool = ctx.enter_context(tc.tile_pool(name="km", bufs=2))
        t1_pool = ctx.enter_context(tc.tile_pool(name="t1", bufs=2))
        im_pool = ctx.enter_context(tc.tile_pool(name="im", bufs=2))
        scr_pool = ctx.enter_context(tc.tile_pool(name="scr", bufs=2))
        p_pool = ctx.enter_context(tc.tile_pool(name="prod", bufs=2))
        mid_pool = ctx.enter_context(tc.tile_pool(name="mid", bufs=2))
        imc_pool = ctx.enter_context(tc.tile_pool(name="imc", bufs=2))
        ex_pool = ctx.enter_context(tc.tile_pool(name="ex", bufs=2))
        ey_pool = ctx.enter_context(tc.tile_pool(name="ey", bufs=2))
        eyim_pool = ctx.enter_context(tc.tile_pool(name="eyim", bufs=2))
        psum_p1 = ctx.enter_context(tc.tile_pool(name="ps1", bufs=2,
                                                 space="PSUM"))
        psum_p2 = ctx.enter_context(tc.tile_pool(name="ps2", bufs=2,
                                                 space="PSUM"))
        psum_pw = ctx.enter_context(tc.tile_pool(name="psw", bufs=2,
                                                 space="PSUM"))

        # ---- constants (fine-grained loads so the first chains start
        # as soon as their chunk arrives) ----
        AS = []
        for s in range(2):
            t = const_pool.tile([128, 5, Ny], bf16, tag=f"AS{s}")
            for c in range(5):
                nc.sync.dma_start(t[:, c, :], ASd.ap()[s, c])
            AS.append(t)
        ASPM = const_pool.tile([128, 5, 128], bf16, tag="ASPM")
        for c in range(5):
            nc.sync.dma_start(ASPM[:, c, :], ASPMd.ap()[c])
        zz = const_pool.tile([1, 512], bf16, tag="zz")
        nc.sync.dma_start(zz[:, :], zzd.ap()[:, :])

        acc = {}
        for bx in range(NTX):
            for comp in range(2):
                t = acc_pool.tile([128, Ny], bf16, tag=f"acc{bx}_{comp}",
                                  name=f"acc{bx}_{comp}")
                acc[(bx, comp)] = t

        # ---- one FFT + coil-combine job ----
        def fft_job(job_id, sm_idx, auxt, first):
            km = km_pool.tile([128, KMW], bf16, tag="km", name=f"km{job_id}")
            nc.sync.dma_start(km[:, :], kmX.ap()[job_id])
            smt = get_sm(sm_idx)
            km5 = km[:, 0:1600].rearrange("p (c x) -> p c x", c=5, x=Ny)

            # pass 1: 4 plain chains + merged x-tail chain -> T1S [128,5,320]
            t1s = t1_pool.tile([128, 5, Ny], bf16, tag="t1s",
                               name=f"t1s{job_id}")
            for (c5, mlo, aset) in ((0, 0, 0), (1, 0, 1),
                                    (2, 128, 0), (3, 128, 1)):
                ps = psum_p1.tile([128, Ny], f32, tag="p1", name="p1")
                for c in range(5):
                    nc.tensor.matmul(ps[:, :], km5[:, c, mlo:mlo + 128],
                                     AS[aset][:, c, :],
                                     start=(c == 0), stop=(c == 4))
                nc.scalar.activation(t1s[:, c5, :], ps[:, :], ACTF.Copy)
            ps = psum_p1.tile([128, Ny], f32, tag="p1", name="p1m2")
            m2lhs = [
                km[:, 1920:2048],
                km[:, 1664:1792],
                km[:, 2048:2176],
                km[:, 1792:1920],
                km[:, 1536:1664],
            ]
            for c in range(5):
                nc.tensor.matmul(ps[:, :], m2lhs[c], AS[0][:, c, :],
                                 start=(c == 0), stop=(c == 4))
            nc.scalar.activation(t1s[:, 4, :], ps[:, :], ACTF.Copy)

            # pass 2: 4 full chains + merged x-tail chain -> im [128,2,3,320]
            imt = im_pool.tile([128, 2, 3, Ny], bf16, tag="im",
                               name=f"im{job_id}")
            for comp in range(2):
                for m in range(2):
                    ps = psum_p2.tile([128, Ny], f32, tag="p2", name="p2")
                    for c in range(5):
                        nc.tensor.matmul(
                            ps[:, :], AS[comp][:, c, 128 * m:128 * m + 128],
                            t1s[:, c, :], start=(c == 0), stop=(c == 4))
                    nc.scalar.activation(imt[:, comp, m, :], ps[:, :],
                                         ACTF.Copy)
            # merged tail: psum rows 0:64 = re x-tail, 64:128 = im x-tail
            ps = psum_p2.tile([128, Ny], f32, tag="p2", name="p2tail")
            for c in range(5):
                nc.tensor.matmul(ps[:, :], ASPM[:, c, :], t1s[:, c, :],
                                 start=(c == 0), stop=(c == 4))
            scr = scr_pool.tile([128, Ny], bf16, tag="scr",
                                name=f"scr{job_id}")
            nc.scalar.activation(scr[:, :], ps[:, :], ACTF.Copy)
            # rows 64:128 of the m2 planes are masked by zeroed smaps but
            # must hold finite values - cover them from scr too.
            nc.sync.dma_start(imt[:, 0, 2, :], scr[:, :])
            nc.sync.dma_start(imt[0:64, 1, 2, :], scr[64:128, :])
            nc.sync.dma_start(imt[64:128, 1, 2, :], scr[64:128, :])

            # coil combine: P = (sr*imr, si*imi, sr*imi, si*imr); real-part
            # accumulation on DVE, imag-part on Pool (engine balance)
            P = p_pool.tile([128, 4, 3, Ny], bf16, tag="P", name=f"P{job_id}")
            nc.vector.tensor_tensor(P[:, 0:2, :, :], smt[:, 0:2, :, :],
                                    imt[:, 0:2, :, :], MULT)
            nc.vector.tensor_tensor(P[:, 2:4, :, :], smt[:, 0:2, :, :],
                                    imt[:, 1::-1, :, :], MULT)
            if first:
                nc.vector.tensor_tensor(auxt[:, 0, :, :], P[:, 0, :, :],
                                        P[:, 1, :, :], ADD)
                nc.gpsimd.tensor_tensor(auxt[:, 1, :, :], P[:, 2, :, :],
                                        P[:, 3, :, :], SUB)
            else:
                nc.vector.tensor_tensor(auxt[:, 0, :, :], auxt[:, 0, :, :],
                                        P[:, 0, :, :], ADD)
                nc.vector.tensor_tensor(auxt[:, 0, :, :], auxt[:, 0, :, :],
                                        P[:, 1, :, :], ADD)
                nc.gpsimd.tensor_tensor(auxt[:, 1, :, :], auxt[:, 1, :, :],
                                        P[:, 2, :, :], ADD)
                nc.gpsimd.tensor_tensor(auxt[:, 1, :, :], auxt[:, 1, :, :],
                                        P[:, 3, :, :], SUB)

        sm_tiles = {}

        def get_sm(idx):
            if idx not in sm_tiles:
                pool = sm_pool if idx < Nc else sm24_pool
                t = pool.tile([128, 2, 3, Ny], bf16,
                              tag=("sm24" if idx >= Nc else f"sm{idx}"),
                              name=f"sm{idx}")
                nc.sync.dma_start(t[:, :, :, :], smX.ap()[idx])
                sm_tiles[idx] = t
            return sm_tiles[idx]

        # ---- warp chunks for one slot ----
        def warp_chunks(slot, auxt, first_acc):
            imc = imc_pool.tile([128, 2, NTX * NTY], bf16, tag="imc",
                                name=f"imc{slot}")

            def prep():
                for comp in range(2):
                    mid = mid_pool.tile([128, 3, 4, NTY], bf16, tag="mid",
                                        name=f"mid{slot}_{comp}")
                    for k in range(3):
                        nc.scalar.activation(
                            mid[:, k, :, :],
                            auxt[:, comp, k, :].rearrange(
                                "p (g ul) -> p g ul", g=NTY, ul=4)
                            .transpose([0, 2, 1]),
                            ACTF.Copy)
                    with nc.allow_non_contiguous_dma(reason="imc gather"):
                        for a in range(NTX):
                            k, a4 = a // 4, a % 4
                            nc.sync.dma_start(
                                imc[:, comp, NTY * a:NTY * a + NTY],
                                mid[32 * a4:32 * a4 + 32, k, :, :]
                                .rearrange("p ul g -> p (ul g)"))

            def mk_bx(bx, emit_out):
                WL = int(WLb[slot, bx])
                JLc = int(JLb[slot, bx])

                def chunk():
                    ext = ex_pool.tile([128, NTY, WL], bf16, tag="ex",
                                       name=f"ex{slot}_{bx}")
                    nc.sync.dma_start(
                        ext[:, :, :],
                        exd.ap()[int(g["ex_off"][slot, bx]):
                                 int(g["ex_off"][slot, bx]) +
                                 128 * NTY * WL]
                        .rearrange("(p t w) -> p t w", p=128, t=NTY, w=WL))
                    eyt = ey_pool.tile([128, JLc, NTY], bf16, tag="ey",
                                       name=f"ey{slot}_{bx}")
                    nc.sync.dma_start(
                        eyt[:, :, :],
                        eyd.ap()[int(g["ey_off"][slot, bx]):
                                 int(g["ey_off"][slot, bx]) +
                                 128 * JLc * NTY]
                        .rearrange("(p j t) -> p j t", p=128, j=JLc, t=NTY))
                    eyim = eyim_pool.tile([128, 2, JLc, NTY], bf16,
                                          tag="eyim",
                                          name=f"eyim{slot}_{bx}")
                    nc.vector.tensor_tensor(
                        eyim[:, :, :, :],
                        eyt.unsqueeze(1).broadcast_to([128, 2, JLc, NTY]),
                        imc[:, :, NTY * bx:NTY * bx + NTY].unsqueeze(2)
                        .broadcast_to([128, 2, JLc, NTY]),
                        MULT)
                    pw = []
                    for comp in range(2):
                        p = psum_pw.tile([128, PWW], f32, tag=f"pw{comp}",
                                         name=f"pw{slot}_{bx}_{comp}")
                        nc.tensor.matmul(p[:, :], zz[0:1, 0:128],
                                         zz[0:1, 0:PWW], start=True,
                                         stop=False, skip_group_check=True)
                        pw.append(p)
                    for ti in range(NTY):
                        obr = int(ox[slot, bx, ti])
                        obc = int(oy[slot, bx, ti])
                        for comp in range(2):
                            nc.tensor.matmul(
                                pw[comp][obr:obr + WL, obc:obc + JLc],
                                ext[:, ti, :], eyim[:, comp, :, ti],
                                start=False, stop=(ti == NTY - 1),
                                skip_group_check=True)
                    for comp in range(2):
                        if first_acc:
                            nc.scalar.activation(
                                acc[(bx, comp)][:, :],
                                pw[comp][:, Dy:Dy + Ny], ACTF.Copy)
                        else:
                            nc.vector.tensor_tensor(
                                acc[(bx, comp)][:, :], acc[(bx, comp)][:, :],
                                pw[comp][:, Dy:Dy + Ny], ADD)
                        if emit_out:
                            nc.sync.dma_start(outp.ap()[bx, comp],
                                              acc[(bx, comp)][:, :])
                return chunk

            emit = (slot == 2)      # last warp streams the outputs
            return [prep] + [mk_bx(bx, emit) for bx in range(NTX)]

        # ---- schedule (slot-major) ----
        aux = {}
        for slot in range(NSLOT):
            aux[slot] = aux_pool.tile([128, 2, 3, Ny], bf16,
                                      tag=f"aux{slot}", name=f"aux{slot}")

        job = 0
        # phase A: slot 0, no warps yet
        for c in range(Nc):
            fft_job(job, c, aux[0], first=(c == 0))
            job += 1
        # phase B: slot 1 || warp of slot 0
        pend = warp_chunks(0, aux[0], first_acc=True)
        pi = 0
        for c in range(Nc):
            fft_job(job, c, aux[1], first=(c == 0))
            job += 1
            goal = ((c + 1) * len(pend) + Nc - 1) // Nc
            while pi < min(goal, len(pend)):
                pend[pi]()
                pi += 1
        while pi < len(pend):
            pend[pi]()
            pi += 1
        # phase C+D: f24 jobs then slot 2 || warps of slots 1 and 3
        pend = warp_chunks(1, aux[1], first_acc=False) + \
            warp_chunks(3, aux[3], first_acc=False)
        pi = 0
        jobs_cd = [(Nc + c, 3, c == 0) for c in range(C24)] + \
            [(c, 2, c == 0) for c in range(Nc)]
        for j, (smi, sl, first) in enumerate(jobs_cd):
            fft_job(job, smi, aux[sl], first)
            job += 1
            goal = ((j + 1) * len(pend) + len(jobs_cd) - 1) // len(jobs_cd)
            while pi < min(goal, len(pend)):
                pend[pi]()
                pi += 1
        assert job == NJOB
        # tail warp: issue its prep before draining leftover warp1/3 chunks
        # so the prep's Act/DMA work hides under their matmuls
        tail_items = warp_chunks(2, aux[2], first_acc=False)
        tail_items[0]()
        while pi < len(pend):
            pend[pi]()
            pi += 1
        for ch in tail_items[1:]:
            ch()

    nc.compile()
    return nc


def _host_prep(kspace_r, kspace_i, mask, smaps_r, smaps_i, flow, g):
    import ml_dtypes
    bf16 = ml_dtypes.bfloat16
    f64 = np.float64
    Dx, Dy = g["Dx"], g["Dy"]
    sx, sy, WLb, JLb = g["sx"], g["sy"], g["WLb"], g["JLb"]

    Ar, Ai = _build_A()
    ASr = _stack5(Ar, -Ai)
    ASi = _stack5(Ai, Ar)
    ASd = np.stack([ASr, ASi]).astype(bf16)
    ASPMd = np.zeros((5, 128, 128), f64)
    ASPMd[:, :, 0:64] = ASr[:, :, 256:320]
    ASPMd[:, :, 64:128] = ASi[:, :, 256:320]
    ASPMd = ASPMd.astype(bf16)

    ksr = kspace_r.astype(f64)
    ksi = kspace_i.astype(f64)
    maskf = mask.astype(f64)

    def jobs_for_core(r):
        fr = [FR_FULL * r + s for s in range(FR_FULL)]
        out = [(c, fr[0]) for c in range(Nc)]
        out += [(c, fr[1]) for c in range(Nc)]
        out += [(C24 * r + c, Nt - 1) for c in range(C24)]
        out += [(c, fr[2]) for c in range(Nc)]
        return out

    # smaps [128, 2 planes (sr, si), 3 m, 320]; m2 rows 64:128 zeroed
    smT = np.zeros((Nc, 128, 2, 3, Ny), f64)
    for c in range(Nc):
        for m in range(3):
            rows = min(128, Nx - 128 * m)
            smT[c, 0:rows, 0, m, :] = smaps_r[128 * m:128 * m + rows, :, c]
            smT[c, 0:rows, 1, m, :] = smaps_i[128 * m:128 * m + rows, :, c]
    smT[:, 64:128, :, 2, :] = 0.0

    # hat tables per frame, packed flat with per-(slot,bx) shapes
    X, Y = np.meshgrid(np.arange(Nx, dtype=f64), np.arange(Ny, dtype=f64),
                       indexing="ij")
    ex_all = {}
    ey_all = {}
    for t in range(Nt):
        slot_t = FR_FULL if t == Nt - 1 else t % FR_FULL
        px = np.clip(X + flow[:, :, 0, t].astype(f64), 0.0, Nx - 1.0)
        py = np.clip(Y + flow[:, :, 1, t].astype(f64), 0.0, Ny - 1.0)
        pxt = px.reshape(NTX, BX, NTY, BY)   # [bx, vl, by, ul]
        pyt = py.reshape(NTX, BX, NTY, BY)
        ex_bx = []
        ey_bx = []
        for bx in range(NTX):
            WL = int(WLb[slot_t, bx])
            JLc = int(JLb[slot_t, bx])
            pxq = pxt[bx].transpose(0, 2, 1).reshape(128, NTY)  # q=4vl+ul
            pyq = pyt[bx].transpose(0, 2, 1).reshape(128, NTY)
            xrow = sx[slot_t, bx].astype(f64)[None, :, None] + \
                np.arange(WL, dtype=f64)[None, None, :]
            ex_bx.append(np.maximum(
                0.0, 1.0 - np.abs(xrow - pxq[:, :, None])
            ).astype(bf16))                                   # [128,NTY,WL]
            yrow = sy[slot_t, bx].astype(f64)[None, :, None] + \
                np.arange(JLc, dtype=f64)[None, None, :]
            eyv = np.maximum(0.0, 1.0 - np.abs(yrow - pyq[:, :, None]))
            ey_bx.append(eyv.astype(bf16).transpose(0, 2, 1))  # [128,JL,NTY]
        ex_all[t] = ex_bx
        ey_all[t] = ey_bx

    zz = np.zeros((1, 512), bf16)

    in_maps = []
    for r in range(NCORES):
        jl = jobs_for_core(r)
        kmX = np.zeros((NJOB, 128, KMW), bf16)
        for ji, (c, t) in enumerate(jl):
            kmr = (ksr[:, :, c] * maskf[:, :, c, t]).T   # [y, x]
            kmi = (ksi[:, :, c] * maskf[:, :, c, t]).T
            kmX[ji] = _km_pack(kmr, kmi).astype(bf16)
        smXc = np.zeros((Nc + C24, 128, 2, 3, Ny), f64)
        smXc[0:Nc] = smT
        for c in range(C24):
            smXc[Nc + c] = smT[C24 * r + c]
        slot_frames = [FR_FULL * r, FR_FULL * r + 1, FR_FULL * r + 2, Nt - 1]
        exd = np.zeros(g["ex_tot"], bf16)
        eyd = np.zeros(g["ey_tot"], bf16)
        for s in range(NSLOT):
            t = slot_frames[s]
            for bx in range(NTX):
                o = int(g["ex_off"][s, bx])
                exd[o:o + ex_all[t][bx].size] = ex_all[t][bx].ravel()
                o = int(g["ey_off"][s, bx])
                eyd[o:o + ey_all[t][bx].size] = ey_all[t][bx].ravel()
        in_maps.append({
            "kmX": kmX,
            "smX": smXc.astype(bf16),
            "ASd": ASd,
            "ASPMd": ASPMd,
            "exd": exd,
            "eyd": eyd,
            "zzd": zz,
        })
    return in_maps


def kernel(kspace_r, kspace_i, mask, smaps_r, smaps_i, flow):
    from concourse.bass_utils import run_bass_kernel_spmd

    flow = np.asarray(flow, np.float32)
    g = _geometry(flow)
    Dx = g["Dx"]

    key = (g["Dx"], g["Dy"], g["sx"].tobytes(), g["sy"].tobytes(),
           g["WLb"].tobytes(), g["JLb"].tobytes())
    if key not in _CACHE:
        _CACHE[key] = _build_program(g)
    nc = _CACHE[key]

    in_maps = _host_prep(
        np.asarray(kspace_r, np.float32), np.asarray(kspace_i, np.float32),
        np.asarray(mask, np.float32), np.asarray(smaps_r, np.float32),
        np.asarray(smaps_i, np.float32), flow, g)

    res = run_bass_kernel_spmd(nc, in_maps, core_ids=list(range(NCORES)))

    out = np.zeros((2, Nx, Ny), np.float64)
    for r in range(NCORES):
        o = res.results[r]["outp"].astype(np.float64)   # [NTX, 2, 128, 320]
        for bx in range(NTX):
            x0 = 32 * bx - Dx
            qlo = max(0, -x0)
            qhi = min(128, Nx - x0, 32 + 2 * Dx)
            for comp in range(2):
                out[comp, x0 + qlo:x0 + qhi, :] += o[bx, comp, qlo:qhi, :]
    return np.stack([out[0], out[1]], axis=-1).astype(np.float32)


# revision 33
# speedup vs baseline: 1.0066x; 1.0066x over previous
# Trainium2 Bass kernel for nn_BatchelorAdj (motion-compensated MRI recon
# adjoint):  out = sum_t W_t^T( sum_c conj(S_c) . IFFT2c(K_c . M_ct) )
#
# v4 design (cost-model driven):
#  - host precomputes km = kspace*mask per (coil,frame) in a stacked-chunk
#    bf16 layout so each IFFT matmul pass runs 5-instruction chains over a
#    640-long stacked contraction ([kmr;kmi] x [A_r;-A_i]); pass 1 merges
#    the 64-row x-tail of the r/i outputs into one 128-row chain, and pass
#    2 merges the two comps' x-tail chains via a stacked ASPM constant
#    (psum rows 0:64 = re tail, 64:128 = im tail) + partition-shift DMA.
#  - smaps: 2 planes (sr, si) resident in SBUF for all 16 coils (+2 f24
#    slots); the second coil-combine product pair uses a negative-stride
#    plane view of the image tile, so no duplicated sr plane is shipped.
#  - adjoint warp: host ships bf16 bilinear hat tables with per-(slot,bx)
#    widths and per-(slot,bx,ti) start offsets that are the union of the 8
#    cores' local windows (shared program => shared offsets), packed flat
#    in DRAM for full-rate DMA.
#  - schedule is slot-major: slot0 jobs; slot1 jobs || warp0; f24 jobs +
#    slot2 jobs || warp1+warp3; tail warp2. Warp table DMA spreads over
#    the whole fft timeline.
#  - per-bx psum bands [128, 324+2D] are flushed into per-bx bf16 SBUF
#    accumulators (Act copy on the first slot, DVE add after); host
#    applies x-shifts in the final cross-core reduce.
import math
import numpy as np

Nx = Ny = 320
Nc = 16
Nt = 25
NCORES = 8
BX, BY = 32, 4
NTX, NTY = Nx // BX, Ny // BY       # 10, 80
FR_FULL = 3
C24 = Nc // NCORES                  # 2
NSLOT = FR_FULL + 1                 # 4
NJOB = FR_FULL * Nc + C24           # 50 fft jobs per core
KMW = 2176                          # km tile width (see _km_pack)

_CACHE = {}


def _build_A():
    j = np.arange(Nx)
    F = np.exp(2j * np.pi * np.outer(j, j) / Nx) / np.sqrt(Nx)
    P = np.zeros((Nx, Nx))
    P[j, (j + Nx // 2) % Nx] = 1.0
    A = P @ F @ P
    return A.real.astype(np.float64), A.imag.astype(np.float64)


def _stack5(M, N):
    """Chunk the 640-row stack [M;N] of two 320-row mats into [5,128,cols]:
    (M0, N0, M1, N1, [M2;N2])."""
    out = np.zeros((5, 128, M.shape[1]), np.float64)
    out[0] = M[0:128]
    out[1] = N[0:128]
    out[2] = M[128:256]
    out[3] = N[128:256]
    out[4, 0:64] = M[256:320]
    out[4, 64:128] = N[256:320]
    return out


def _km_pack(kmr, kmi):
    """km [y,x] pair -> [128, 2176] layout.
    cols [0:1600]:   km5 chunks (kmr0,kmi0,kmr1,kmi1,[kmr2;kmi2]) full x.
    cols [1600:1664]: [kmi2 ; -kmr2] @ x 256:320 (partition-stacked)
    cols [1664:1920]: (kmi-y0, -kmr-y0, kmi-y1, -kmr-y1) @ x 256:320
    cols [1920:2176]: (kmr-y0, kmi-y0, kmr-y1, kmi-y1) @ x 256:320."""
    out = np.zeros((128, KMW), np.float64)
    km5 = _stack5(kmr, kmi)
    out[:, 0:1600] = km5.transpose(1, 0, 2).reshape(128, 1600)
    out[0:64, 1600:1664] = kmi[256:320, 256:320]
    out[64:128, 1600:1664] = -kmr[256:320, 256:320]
    out[:, 1664:1728] = kmi[0:128, 256:320]
    out[:, 1728:1792] = -kmr[0:128, 256:320]
    out[:, 1792:1856] = kmi[128:256, 256:320]
    out[:, 1856:1920] = -kmr[128:256, 256:320]
    out[:, 1920:1984] = kmr[0:128, 256:320]
    out[:, 1984:2048] = kmi[0:128, 256:320]
    out[:, 2048:2112] = kmr[128:256, 256:320]
    out[:, 2112:2176] = kmi[128:256, 256:320]
    return out


def _geometry(flow):
    """Union-local warp windows shared by all 8 cores.

    Returns dict with per-(slot,bx,ti) y/x window starts (sy, sx), per-
    (slot,bx) widths (JLb, WLb), flat ex/ey table offsets, and Dx/Dy."""
    f64 = np.float64
    Dx = int(math.ceil(np.abs(flow[:, :, 0, :]).max()))
    Dy = int(math.ceil(np.abs(flow[:, :, 1, :]).max()))
    W = 32 + 2 * Dx
    Xg, Yg = np.meshgrid(np.arange(Nx, dtype=f64), np.arange(Ny, dtype=f64),
                         indexing="ij")
    fminx = np.zeros((Nt, NTX, NTY), np.int64)
    fmaxx = np.zeros((Nt, NTX, NTY), np.int64)
    fminy = np.zeros((Nt, NTX, NTY), np.int64)
    fmaxy = np.zeros((Nt, NTX, NTY), np.int64)
    for t in range(Nt):
        px = np.clip(Xg + flow[:, :, 0, t].astype(f64), 0.0, Nx - 1.0)
        py = np.clip(Yg + flow[:, :, 1, t].astype(f64), 0.0, Ny - 1.0)
        fx = np.floor(px).astype(np.int64).reshape(NTX, BX, NTY, BY)
        fy = np.floor(py).astype(np.int64).reshape(NTX, BX, NTY, BY)
        fminx[t] = fx.min(axis=(1, 3))
        fmaxx[t] = fx.max(axis=(1, 3))
        fminy[t] = fy.min(axis=(1, 3))
        fmaxy[t] = fy.max(axis=(1, 3))

    sx = np.zeros((NSLOT, NTX, NTY), np.int64)
    ex_hi = np.zeros((NSLOT, NTX, NTY), np.int64)
    sy = np.zeros((NSLOT, NTX, NTY), np.int64)
    ey_hi = np.zeros((NSLOT, NTX, NTY), np.int64)
    for s in range(FR_FULL):
        frames = [FR_FULL * r + s for r in range(NCORES)]
        sx[s] = fminx[frames].min(axis=0)
        ex_hi[s] = fmaxx[frames].max(axis=0)
        sy[s] = fminy[frames].min(axis=0)
        ey_hi[s] = fmaxy[frames].max(axis=0)
    sx[FR_FULL], ex_hi[FR_FULL] = fminx[Nt - 1], fmaxx[Nt - 1]
    sy[FR_FULL], ey_hi[FR_FULL] = fminy[Nt - 1], fmaxy[Nt - 1]

    # x windows: PE psum writes need base partition in {0,32,64}, so pin
    # the window start at the band start and trim only the upper edge.
    bxlo = (32 * np.arange(NTX) - Dx)[None, :, None]
    sx = np.broadcast_to(bxlo, sx.shape).copy()
    WLb = np.minimum((ex_hi - sx + 2).max(axis=2), W)   # [NSLOT, NTX]
    JLb = (ey_hi - sy + 2).max(axis=2)
    sy = np.minimum(sy, Ny + Dy + 4 - JLb[:, :, None])

    ex_off = np.zeros((NSLOT, NTX), np.int64)
    ey_off = np.zeros((NSLOT, NTX), np.int64)
    pos = 0
    for s in range(NSLOT):
        for bx in range(NTX):
            ex_off[s, bx] = pos
            pos += 128 * NTY * int(WLb[s, bx])
    ex_tot = pos
    pos = 0
    for s in range(NSLOT):
        for bx in range(NTX):
            ey_off[s, bx] = pos
            pos += 128 * int(JLb[s, bx]) * NTY
    ey_tot = pos
    return dict(Dx=Dx, Dy=Dy, W=W, sx=sx, sy=sy, WLb=WLb, JLb=JLb,
                ex_off=ex_off, ey_off=ey_off, ex_tot=ex_tot, ey_tot=ey_tot)


def _build_program(g):
    from concourse import bass, bacc, tile, mybir

    Dx, Dy, W = g["Dx"], g["Dy"], g["W"]
    PWW = Ny + 2 * Dy + 4              # psum band columns (y + Dy offset)
    oy = g["sy"] + Dy                  # psum band col starts
    ox = g["sx"] - (32 * np.arange(NTX) - Dx)[None, :, None]  # band rows
    WLb, JLb = g["WLb"], g["JLb"]
    f32 = mybir.dt.float32
    bf16 = mybir.dt.bfloat16
    MULT = mybir.AluOpType.mult
    ADD = mybir.AluOpType.add
    SUB = mybir.AluOpType.subtract
    ACTF = mybir.ActivationFunctionType

    nc = bacc.Bacc("TRN2", target_bir_lowering=False, debug=False,
                   num_devices=NCORES)

    kmX = nc.dram_tensor("kmX", [NJOB, 128, KMW], bf16, kind="ExternalInput")
    smX = nc.dram_tensor("smX", [Nc + C24, 128, 2, 3, Ny], bf16,
                         kind="ExternalInput")
    ASd = nc.dram_tensor("ASd", [2, 5, 128, Ny], bf16, kind="ExternalInput")
    ASPMd = nc.dram_tensor("ASPMd", [5, 128, 128], bf16,
                           kind="ExternalInput")
    exd = nc.dram_tensor("exd", [g["ex_tot"]], bf16, kind="ExternalInput")
    eyd = nc.dram_tensor("eyd", [g["ey_tot"]], bf16, kind="ExternalInput")
    zzd = nc.dram_tensor("zzd", [1, 512], bf16, kind="ExternalInput")
    outp = nc.dram_tensor("outp", [NTX, 2, 128, Ny], bf16,
                          kind="ExternalOutput")

    from contextlib import ExitStack
    with tile.TileContext(nc) as tc, ExitStack() as ctx:
        const_pool = ctx.enter_context(tc.tile_pool(name="const", bufs=1))
        acc_pool = ctx.enter_context(tc.tile_pool(name="acc", bufs=1))
        aux_pool = ctx.enter_context(tc.tile_pool(name="aux", bufs=1))
        sm_pool = ctx.enter_context(tc.tile_pool(name="sm", bufs=1))
        sm24_pool = ctx.enter_context(tc.tile_pool(name="sm24", bufs=2))
        km_pool = ctx.enter_context(tc.tile_pool(name="km", bufs=2))
        t1_pool = ctx.enter_context(tc.tile_pool(name="t1", bufs=2))
        im_pool = ctx.enter_context(tc.tile_pool(name="im", bufs=2))
        scr_pool = ctx.enter_context(tc.tile_pool(name="scr", bufs=2))
        p_pool = ctx.enter_context(tc.tile_pool(name="prod", bufs=2))
        mid_pool = ctx.enter_context(tc.tile_pool(name="mid", bufs=2))
        imc_pool = ctx.enter_context(tc.tile_pool(name="imc", bufs=2))
        ex_pool = ctx.enter_context(tc.tile_pool(name="ex", bufs=2))
        ey_pool = ctx.enter_context(tc.tile_pool(name="ey", bufs=2))
        eyim_pool = ctx.enter_context(tc.tile_pool(name="eyim", bufs=2))
        psum_p1 = ctx.enter_context(tc.tile_pool(name="ps1", bufs=2,
                                                 space="PSUM"))
        psum_p2 = ctx.enter_context(tc.tile_pool(name="ps2", bufs=2,
                                                 space="PSUM"))
        psum_pw = ctx.enter_context(tc.tile_pool(name="psw", bufs=2,
                                                 space="PSUM"))

        # ---- constants (fine-grained loads so the first chains start
        # as soon as their chunk arrives) ----
        AS = []
        for s in range(2):
            t = const_pool.tile([128, 5, Ny], bf16, tag=f"AS{s}")
            for c in range(5):
                nc.sync.dma_start(t[:, c, :], ASd.ap()[s, c])
            AS.append(t)
        ASPM = const_pool.tile([128, 5, 128], bf16, tag="ASPM")
        for c in range(5):
            nc.sync.dma_start(ASPM[:, c, :], ASPMd.ap()[c])
        zz = const_pool.tile([1, 512], bf16, tag="zz")
        nc.sync.dma_start(zz[:, :], zzd.ap()[:, :])

        acc = {}
        for bx in range(NTX):
            for comp in range(2):
                t = acc_pool.tile([128, Ny], bf16, tag=f"acc{bx}_{comp}",
                                  name=f"acc{bx}_{comp}")
                acc[(bx, comp)] = t

        # ---- one FFT + coil-combine job ----
        def fft_job(job_id, sm_idx, auxt, first):
            km = km_pool.tile([128, KMW], bf16, tag="km", name=f"km{job_id}")
            nc.sync.dma_start(km[:, :], kmX.ap()[job_id])
            smt = get_sm(sm_idx)
            km5 = km[:, 0:1600].rearrange("p (c x) -> p c x", c=5, x=Ny)

            # pass 1: 4 plain chains + merged x-tail chain -> T1S [128,5,320]
            t1s = t1_pool.tile([128, 5, Ny], bf16, tag="t1s",
                               name=f"t1s{job_id}")
            for (c5, mlo, aset) in ((0, 0, 0), (1, 0, 1),
                                    (2, 128, 0), (3, 128, 1)):
                ps = psum_p1.tile([128, Ny], f32, tag="p1", name="p1")
                for c in range(5):
                    nc.tensor.matmul(ps[:, :], km5[:, c, mlo:mlo + 128],
                                     AS[aset][:, c, :],
                                     start=(c == 0), stop=(c == 4))
                nc.scalar.activation(t1s[:, c5, :], ps[:, :], ACTF.Copy)
            ps = psum_p1.tile([128, Ny], f32, tag="p1", name="p1m2")
            m2lhs = [
                km[:, 1920:2048],
                km[:, 1664:1792],
                km[:, 2048:2176],
                km[:, 1792:1920],
                km[:, 1536:1664],
            ]
            for c in range(5):
                nc.tensor.matmul(ps[:, :], m2lhs[c], AS[0][:, c, :],
                                 start=(c == 0), stop=(c == 4))
            nc.scalar.activation(t1s[:, 4, :], ps[:, :], ACTF.Copy)

            # pass 2: 4 full chains + merged x-tail chain -> im [128,2,3,320]
            imt = im_pool.tile([128, 2, 3, Ny], bf16, tag="im",
                               name=f"im{job_id}")
            for comp in range(2):
                for m in range(2):
                    ps = psum_p2.tile([128, Ny], f32, tag="p2", name="p2")
                    for c in range(5):
                        nc.tensor.matmul(
                            ps[:, :], AS[comp][:, c, 128 * m:128 * m + 128],
                            t1s[:, c, :], start=(c == 0), stop=(c == 4))
                    nc.scalar.activation(imt[:, comp, m, :], ps[:, :],
                                         ACTF.Copy)
            # merged tail: psum rows 0:64 = re x-tail, 64:128 = im x-tail
            ps = psum_p2.tile([128, Ny], f32, tag="p2", name="p2tail")
            for c in range(5):
                nc.tensor.matmul(ps[:, :], ASPM[:, c, :], t1s[:, c, :],
                                 start=(c == 0), stop=(c == 4))
            scr = scr_pool.tile([128, Ny], bf16, tag="scr",
                                name=f"scr{job_id}")
            nc.scalar.activation(scr[:, :], ps[:, :], ACTF.Copy)
            # rows 64:128 of the m2 planes are masked by zeroed smaps but
            # must hold finite values - cover them from scr too.
            nc.sync.dma_start(imt[:, 0, 2, :], scr[:, :])
            nc.sync.dma_start(imt[0:64, 1, 2, :], scr[64:128, :])
            nc.sync.dma_start(imt[64:128, 1, 2, :], scr[64:128, :])

            # coil combine: P = (sr*imr, si*imi, sr*imi, si*imr); real-part
            # accumulation on DVE, imag-part on Pool (engine balance)
            P = p_pool.tile([128, 4, 3, Ny], bf16, tag="P", name=f"P{job_id}")
            nc.vector.tensor_tensor(P[:, 0:2, :, :], smt[:, 0:2, :, :],
                                    imt[:, 0:2, :, :], MULT)
            nc.vector.tensor_tensor(P[:, 2:4, :, :], smt[:, 0:2, :, :],
                                    imt[:, 1::-1, :, :], MULT)
            if first:
                nc.vector.tensor_tensor(auxt[:, 0, :, :], P[:, 0, :, :],
                                        P[:, 1, :, :], ADD)
                nc.gpsimd.tensor_tensor(auxt[:, 1, :, :], P[:, 2, :, :],
                                        P[:, 3, :, :], SUB)
            else:
                nc.vector.tensor_tensor(auxt[:, 0, :, :], auxt[:, 0, :, :],
                                        P[:, 0, :, :], ADD)
                nc.vector.tensor_tensor(auxt[:, 0, :, :], auxt[:, 0, :, :],
                                        P[:, 1, :, :], ADD)
                nc.gpsimd.tensor_tensor(auxt[:, 1, :, :], auxt[:, 1, :, :],
                                        P[:, 2, :, :], ADD)
                nc.gpsimd.tensor_tensor(auxt[:, 1, :, :], auxt[:, 1, :, :],
                                        P[:, 3, :, :], SUB)

        sm_tiles = {}

        def get_sm(idx):
            if idx not in sm_tiles:
                pool = sm_pool if idx < Nc else sm24_pool
                t = pool.tile([128, 2, 3, Ny], bf16,
                              tag=("sm24" if idx >= Nc else f"sm{idx}"),
                              name=f"sm{idx}")
                nc.sync.dma_start(t[:, :, :, :], smX.ap()[idx])
                sm_tiles[idx] = t
            return sm_tiles[idx]

        # ---- warp chunks for one slot ----
        def warp_chunks(slot, auxt, first_acc):
            imc = imc_pool.tile([128, 2, NTX * NTY], bf16, tag="imc",
                                name=f"imc{slot}")

            def prep():
                for comp in range(2):
                    mid = mid_pool.tile([128, 3, 4, NTY], bf16, tag="mid",
                                        name=f"mid{slot}_{comp}")
                    for k in range(3):
                        nc.scalar.activation(
                            mid[:, k, :, :],
                            auxt[:, comp, k, :].rearrange(
                                "p (g ul) -> p g ul", g=NTY, ul=4)
                            .transpose([0, 2, 1]),
                            ACTF.Copy)
                    with nc.allow_non_contiguous_dma(reason="imc gather"):
                        for a in range(NTX):
                            k, a4 = a // 4, a % 4
                            nc.sync.dma_start(
                                imc[:, comp, NTY * a:NTY * a + NTY],
                                mid[32 * a4:32 * a4 + 32, k, :, :]
                                .rearrange("p ul g -> p (ul g)"))

            def mk_bx(bx):
                WL = int(WLb[slot, bx])
                JLc = int(JLb[slot, bx])

                def chunk():
                    ext = ex_pool.tile([128, NTY, WL], bf16, tag="ex",
                                       name=f"ex{slot}_{bx}")
                    nc.sync.dma_start(
                        ext[:, :, :],
                        exd.ap()[int(g["ex_off"][slot, bx]):
                                 int(g["ex_off"][slot, bx]) +
                                 128 * NTY * WL]
                        .rearrange("(p t w) -> p t w", p=128, t=NTY, w=WL))
                    eyt = ey_pool.tile([128, JLc, NTY], bf16, tag="ey",
                                       name=f"ey{slot}_{bx}")
                    nc.sync.dma_start(
                        eyt[:, :, :],
                        eyd.ap()[int(g["ey_off"][slot, bx]):
                                 int(g["ey_off"][slot, bx]) +
                                 128 * JLc * NTY]
                        .rearrange("(p j t) -> p j t", p=128, j=JLc, t=NTY))
                    eyim = eyim_pool.tile([128, 2, JLc, NTY], bf16,
                                          tag="eyim",
                                          name=f"eyim{slot}_{bx}")
                    nc.vector.tensor_tensor(
                        eyim[:, :, :, :],
                        eyt.unsqueeze(1).broadcast_to([128, 2, JLc, NTY]),
                        imc[:, :, NTY * bx:NTY * bx + NTY].unsqueeze(2)
                        .broadcast_to([128, 2, JLc, NTY]),
                        MULT)
                    pw = []
                    for comp in range(2):
                        p = psum_pw.tile([128, PWW], f32, tag=f"pw{comp}",
                                         name=f"pw{slot}_{bx}_{comp}")
                        nc.tensor.matmul(p[:, :], zz[0:1, 0:128],
                                         zz[0:1, 0:PWW], start=True,
                                         stop=False, skip_group_check=True)
                        pw.append(p)
                    for ti in range(NTY):
                        obr = int(ox[slot, bx, ti])
                        obc = int(oy[slot, bx, ti])
                        for comp in range(2):
                            nc.tensor.matmul(
                                pw[comp][obr:obr + WL, obc:obc + JLc],
                                ext[:, ti, :], eyim[:, comp, :, ti],
                                start=False, stop=(ti == NTY - 1),
                                skip_group_check=True)
                    for comp in range(2):
                        if first_acc:
                            nc.scalar.activation(
                                acc[(bx, comp)][:, :],
                                pw[comp][:, Dy:Dy + Ny], ACTF.Copy)
                        else:
                            nc.vector.tensor_tensor(
                                acc[(bx, comp)][:, :], acc[(bx, comp)][:, :],
                                pw[comp][:, Dy:Dy + Ny], ADD)
                return chunk

            return [prep] + [mk_bx(bx) for bx in range(NTX)]

        # ---- schedule (slot-major) ----
        aux = {}
        for slot in range(NSLOT):
            aux[slot] = aux_pool.tile([128, 2, 3, Ny], bf16,
                                      tag=f"aux{slot}", name=f"aux{slot}")

        job = 0
        # phase A: slot 0, no warps yet
        for c in range(Nc):
            fft_job(job, c, aux[0], first=(c == 0))
            job += 1
        # phase B: slot 1 || warp of slot 0
        pend = warp_chunks(0, aux[0], first_acc=True)
        pi = 0
        for c in range(Nc):
            fft_job(job, c, aux[1], first=(c == 0))
            job += 1
            goal = ((c + 1) * len(pend) + Nc - 1) // Nc
            while pi < min(goal, len(pend)):
                pend[pi]()
                pi += 1
        while pi < len(pend):
            pend[pi]()
            pi += 1
        # phase C+D: f24 jobs then slot 2 || warps of slots 1 and 3
        pend = warp_chunks(1, aux[1], first_acc=False) + \
            warp_chunks(3, aux[3], first_acc=False)
        pi = 0
        jobs_cd = [(Nc + c, 3, c == 0) for c in range(C24)] + \
            [(c, 2, c == 0) for c in range(Nc)]
        for j, (smi, sl, first) in enumerate(jobs_cd):
            fft_job(job, smi, aux[sl], first)
            job += 1
            goal = ((j + 1) * len(pend) + len(jobs_cd) - 1) // len(jobs_cd)
            while pi < min(goal, len(pend)):
                pend[pi]()
                pi += 1
        while pi < len(pend):
            pend[pi]()
            pi += 1
        assert job == NJOB
        for ch in warp_chunks(2, aux[2], first_acc=False):  # tail warp
            ch()

        for bx in range(NTX):
            for comp in range(2):
                nc.sync.dma_start(outp.ap()[bx, comp],
                                  acc[(bx, comp)][:, :])

    nc.compile()
    return nc


def _host_prep(kspace_r, kspace_i, mask, smaps_r, smaps_i, flow, g):
    import ml_dtypes
    bf16 = ml_dtypes.bfloat16
    f64 = np.float64
    Dx, Dy = g["Dx"], g["Dy"]
    sx, sy, WLb, JLb = g["sx"], g["sy"], g["WLb"], g["JLb"]

    Ar, Ai = _build_A()
    ASr = _stack5(Ar, -Ai)
    ASi = _stack5(Ai, Ar)
    ASd = np.stack([ASr, ASi]).astype(bf16)
    ASPMd = np.zeros((5, 128, 128), f64)
    ASPMd[:, :, 0:64] = ASr[:, :, 256:320]
    ASPMd[:, :, 64:128] = ASi[:, :, 256:320]
    ASPMd = ASPMd.astype(bf16)

    ksr = kspace_r.astype(f64)
    ksi = kspace_i.astype(f64)
    maskf = mask.astype(f64)

    def jobs_for_core(r):
        fr = [FR_FULL * r + s for s in range(FR_FULL)]
        out = [(c, fr[0]) for c in range(Nc)]
        out += [(c, fr[1]) for c in range(Nc)]
        out += [(C24 * r + c, Nt - 1) for c in range(C24)]
        out += [(c, fr[2]) for c in range(Nc)]
        return out

    # smaps [128, 2 planes (sr, si), 3 m, 320]; m2 rows 64:128 zeroed
    smT = np.zeros((Nc, 128, 2, 3, Ny), f64)
    for c in range(Nc):
        for m in range(3):
            rows = min(128, Nx - 128 * m)
            smT[c, 0:rows, 0, m, :] = smaps_r[128 * m:128 * m + rows, :, c]
            smT[c, 0:rows, 1, m, :] = smaps_i[128 * m:128 * m + rows, :, c]
    smT[:, 64:128, :, 2, :] = 0.0

    # hat tables per frame, packed flat with per-(slot,bx) shapes
    X, Y = np.meshgrid(np.arange(Nx, dtype=f64), np.arange(Ny, dtype=f64),
                       indexing="ij")
    ex_all = {}
    ey_all = {}
    for t in range(Nt):
        slot_t = FR_FULL if t == Nt - 1 else t % FR_FULL
        px = np.clip(X + flow[:, :, 0, t].astype(f64), 0.0, Nx - 1.0)
        py = np.clip(Y + flow[:, :, 1, t].astype(f64), 0.0, Ny - 1.0)
        pxt = px.reshape(NTX, BX, NTY, BY)   # [bx, vl, by, ul]
        pyt = py.reshape(NTX, BX, NTY, BY)
        ex_bx = []
        ey_bx = []
        for bx in range(NTX):
            WL = int(WLb[slot_t, bx])
            JLc = int(JLb[slot_t, bx])
            pxq = pxt[bx].transpose(0, 2, 1).reshape(128, NTY)  # q=4vl+ul
            pyq = pyt[bx].transpose(0, 2, 1).reshape(128, NTY)
            xrow = sx[slot_t, bx].astype(f64)[None, :, None] + \
                np.arange(WL, dtype=f64)[None, None, :]
            ex_bx.append(np.maximum(
                0.0, 1.0 - np.abs(xrow - pxq[:, :, None])
            ).astype(bf16))                                   # [128,NTY,WL]
            yrow = sy[slot_t, bx].astype(f64)[None, :, None] + \
                np.arange(JLc, dtype=f64)[None, None, :]
            eyv = np.maximum(0.0, 1.0 - np.abs(yrow - pyq[:, :, None]))
            ey_bx.append(eyv.astype(bf16).transpose(0, 2, 1))  # [128,JL,NTY]
        ex_all[t] = ex_bx
        ey_all[t] = ey_bx

    zz = np.zeros((1, 512), bf16)

    in_maps = []
    for r in range(NCORES):
        jl = jobs_for_core(r)
        kmX = np.zeros((NJOB, 128, KMW), bf16)
        for ji, (c, t) in enumerate(jl):
            kmr = (ksr[:, :, c] * maskf[:, :, c, t]).T   # [y, x]
            kmi = (ksi[:, :, c] * maskf[:, :, c, t]).T
            kmX[ji] = _km_pack(kmr, kmi).astype(bf16)
        smXc = np.zeros((Nc + C24, 128, 2, 3, Ny), f64)
        smXc[0:Nc] = smT
        for c in range(C24):
            smXc[Nc + c] = smT[C24 * r + c]
        slot_frames = [FR_FULL * r, FR_FULL * r + 1, FR_FULL * r + 2, Nt - 1]
        exd = np.zeros(g["ex_tot"], bf16)
        eyd = np.zeros(g["ey_tot"], bf16)
        for s in range(NSLOT):
            t = slot_frames[s]
            for bx in range(NTX):
                o = int(g["ex_off"][s, bx])
                exd[o:o + ex_all[t][bx].size] = ex_all[t][bx].ravel()
                o = int(g["ey_off"][s, bx])
                eyd[o:o + ey_all[t][bx].size] = ey_all[t][bx].ravel()
        in_maps.append({
            "kmX": kmX,
            "smX": smXc.astype(bf16),
            "ASd": ASd,
            "ASPMd": ASPMd,
            "exd": exd,
            "eyd": eyd,
            "zzd": zz,
        })
    return in_maps


def kernel(kspace_r, kspace_i, mask, smaps_r, smaps_i, flow):
    from concourse.bass_utils import run_bass_kernel_spmd

    flow = np.asarray(flow, np.float32)
    g = _geometry(flow)
    Dx = g["Dx"]

    key = (g["Dx"], g["Dy"], g["sx"].tobytes(), g["sy"].tobytes(),
           g["WLb"].tobytes(), g["JLb"].tobytes())
    if key not in _CACHE:
        _CACHE[key] = _build_program(g)
    nc = _CACHE[key]

    in_maps = _host_prep(
        np.asarray(kspace_r, np.float32), np.asarray(kspace_i, np.float32),
        np.asarray(mask, np.float32), np.asarray(smaps_r, np.float32),
        np.asarray(smaps_i, np.float32), flow, g)

    res = run_bass_kernel_spmd(nc, in_maps, core_ids=list(range(NCORES)))

    out = np.zeros((2, Nx, Ny), np.float64)
    for r in range(NCORES):
        o = res.results[r]["outp"].astype(np.float64)   # [NTX, 2, 128, 320]
        for bx in range(NTX):
            x0 = 32 * bx - Dx
            qlo = max(0, -x0)
            qhi = min(128, Nx - x0, 32 + 2 * Dx)
            for comp in range(2):
                out[comp, x0 + qlo:x0 + qhi, :] += o[bx, comp, qlo:qhi, :]
    return np.stack([out[0], out[1]], axis=-1).astype(np.float32)


# revision 34
# speedup vs baseline: 1.0196x; 1.0129x over previous
# Trainium2 Bass kernel for nn_BatchelorAdj (motion-compensated MRI recon
# adjoint):  out = sum_t W_t^T( sum_c conj(S_c) . IFFT2c(K_c . M_ct) )
#
# v4 design (cost-model driven):
#  - host precomputes km = kspace*mask per (coil,frame) in a stacked-chunk
#    bf16 layout so each IFFT matmul pass runs 5-instruction chains over a
#    640-long stacked contraction ([kmr;kmi] x [A_r;-A_i]); pass 1 merges
#    the 64-row x-tail of the r/i outputs into one 128-row chain, and pass
#    2 merges the two comps' x-tail chains via a stacked ASPM constant
#    (psum rows 0:64 = re tail, 64:128 = im tail) + partition-shift DMA.
#  - smaps: 2 planes (sr, si) resident in SBUF for all 16 coils (+2 f24
#    slots); the second coil-combine product pair uses a negative-stride
#    plane view of the image tile, so no duplicated sr plane is shipped.
#  - adjoint warp: host ships bf16 bilinear hat tables with per-(slot,bx)
#    widths and per-(slot,bx,ti) start offsets that are the union of the 8
#    cores' local windows (shared program => shared offsets), packed flat
#    in DRAM for full-rate DMA.
#  - schedule is slot-major: slot0 jobs; slot1 jobs || warp0; f24 jobs +
#    slot2 jobs || warp1+warp3; tail warp2. Warp table DMA spreads over
#    the whole fft timeline.
#  - per-bx psum bands [128, 324+2D] are flushed into per-bx bf16 SBUF
#    accumulators (Act copy on the first slot, DVE add after); host
#    applies x-shifts in the final cross-core reduce.
import math
import numpy as np

Nx = Ny = 320
Nc = 16
Nt = 25
NCORES = 8
BX, BY = 32, 4
NTX, NTY = Nx // BX, Ny // BY       # 10, 80
FR_FULL = 3
C24 = Nc // NCORES                  # 2
NSLOT = FR_FULL + 1                 # 4
NJOB = FR_FULL * Nc + C24           # 50 fft jobs per core
KMW = 2176                          # km tile width (see _km_pack)

_CACHE = {}


def _build_A():
    j = np.arange(Nx)
    F = np.exp(2j * np.pi * np.outer(j, j) / Nx) / np.sqrt(Nx)
    P = np.zeros((Nx, Nx))
    P[j, (j + Nx // 2) % Nx] = 1.0
    A = P @ F @ P
    return A.real.astype(np.float64), A.imag.astype(np.float64)


def _stack5(M, N):
    """Chunk the 640-row stack [M;N] of two 320-row mats into [5,128,cols]:
    (M0, N0, M1, N1, [M2;N2])."""
    out = np.zeros((5, 128, M.shape[1]), np.float64)
    out[0] = M[0:128]
    out[1] = N[0:128]
    out[2] = M[128:256]
    out[3] = N[128:256]
    out[4, 0:64] = M[256:320]
    out[4, 64:128] = N[256:320]
    return out


def _km_pack(kmr, kmi):
    """km [y,x] pair -> [128, 2176] layout.
    cols [0:1600]:   km5 chunks (kmr0,kmi0,kmr1,kmi1,[kmr2;kmi2]) full x.
    cols [1600:1664]: [kmi2 ; -kmr2] @ x 256:320 (partition-stacked)
    cols [1664:1920]: (kmi-y0, -kmr-y0, kmi-y1, -kmr-y1) @ x 256:320
    cols [1920:2176]: (kmr-y0, kmi-y0, kmr-y1, kmi-y1) @ x 256:320."""
    out = np.zeros((128, KMW), np.float64)
    km5 = _stack5(kmr, kmi)
    out[:, 0:1600] = km5.transpose(1, 0, 2).reshape(128, 1600)
    out[0:64, 1600:1664] = kmi[256:320, 256:320]
    out[64:128, 1600:1664] = -kmr[256:320, 256:320]
    out[:, 1664:1728] = kmi[0:128, 256:320]
    out[:, 1728:1792] = -kmr[0:128, 256:320]
    out[:, 1792:1856] = kmi[128:256, 256:320]
    out[:, 1856:1920] = -kmr[128:256, 256:320]
    out[:, 1920:1984] = kmr[0:128, 256:320]
    out[:, 1984:2048] = kmi[0:128, 256:320]
    out[:, 2048:2112] = kmr[128:256, 256:320]
    out[:, 2112:2176] = kmi[128:256, 256:320]
    return out


def _geometry(flow):
    """Union-local warp windows shared by all 8 cores.

    Returns dict with per-(slot,bx,ti) y/x window starts (sy, sx), per-
    (slot,bx) widths (JLb, WLb), flat ex/ey table offsets, and Dx/Dy."""
    f64 = np.float64
    Dx = int(math.ceil(np.abs(flow[:, :, 0, :]).max()))
    Dy = int(math.ceil(np.abs(flow[:, :, 1, :]).max()))
    W = 32 + 2 * Dx
    Xg, Yg = np.meshgrid(np.arange(Nx, dtype=f64), np.arange(Ny, dtype=f64),
                         indexing="ij")
    fminx = np.zeros((Nt, NTX, NTY), np.int64)
    fmaxx = np.zeros((Nt, NTX, NTY), np.int64)
    fminy = np.zeros((Nt, NTX, NTY), np.int64)
    fmaxy = np.zeros((Nt, NTX, NTY), np.int64)
    for t in range(Nt):
        px = np.clip(Xg + flow[:, :, 0, t].astype(f64), 0.0, Nx - 1.0)
        py = np.clip(Yg + flow[:, :, 1, t].astype(f64), 0.0, Ny - 1.0)
        fx = np.floor(px).astype(np.int64).reshape(NTX, BX, NTY, BY)
        fy = np.floor(py).astype(np.int64).reshape(NTX, BX, NTY, BY)
        fminx[t] = fx.min(axis=(1, 3))
        fmaxx[t] = fx.max(axis=(1, 3))
        fminy[t] = fy.min(axis=(1, 3))
        fmaxy[t] = fy.max(axis=(1, 3))

    sx = np.zeros((NSLOT, NTX, NTY), np.int64)
    ex_hi = np.zeros((NSLOT, NTX, NTY), np.int64)
    sy = np.zeros((NSLOT, NTX, NTY), np.int64)
    ey_hi = np.zeros((NSLOT, NTX, NTY), np.int64)
    for s in range(FR_FULL):
        frames = [FR_FULL * r + s for r in range(NCORES)]
        sx[s] = fminx[frames].min(axis=0)
        ex_hi[s] = fmaxx[frames].max(axis=0)
        sy[s] = fminy[frames].min(axis=0)
        ey_hi[s] = fmaxy[frames].max(axis=0)
    sx[FR_FULL], ex_hi[FR_FULL] = fminx[Nt - 1], fmaxx[Nt - 1]
    sy[FR_FULL], ey_hi[FR_FULL] = fminy[Nt - 1], fmaxy[Nt - 1]

    # x windows: PE psum writes need base partition in {0,32,64}, so pin
    # the window start at the band start and trim only the upper edge.
    bxlo = (32 * np.arange(NTX) - Dx)[None, :, None]
    sx = np.broadcast_to(bxlo, sx.shape).copy()
    WLb = np.minimum((ex_hi - sx + 2).max(axis=2), W)   # [NSLOT, NTX]
    JLb = (ey_hi - sy + 2).max(axis=2)
    sy = np.minimum(sy, Ny + Dy + 4 - JLb[:, :, None])

    ex_off = np.zeros((NSLOT, NTX), np.int64)
    ey_off = np.zeros((NSLOT, NTX), np.int64)
    pos = 0
    for s in range(NSLOT):
        for bx in range(NTX):
            ex_off[s, bx] = pos
            pos += 128 * NTY * int(WLb[s, bx])
    ex_tot = pos
    pos = 0
    for s in range(NSLOT):
        for bx in range(NTX):
            ey_off[s, bx] = pos
            pos += 128 * int(JLb[s, bx]) * NTY
    ey_tot = pos
    return dict(Dx=Dx, Dy=Dy, W=W, sx=sx, sy=sy, WLb=WLb, JLb=JLb,
                ex_off=ex_off, ey_off=ey_off, ex_tot=ex_tot, ey_tot=ey_tot)


def _build_program(g):
    from concourse import bass, bacc, tile, mybir

    Dx, Dy, W = g["Dx"], g["Dy"], g["W"]
    PWW = Ny + 2 * Dy + 4              # psum band columns (y + Dy offset)
    oy = g["sy"] + Dy                  # psum band col starts
    ox = g["sx"] - (32 * np.arange(NTX) - Dx)[None, :, None]  # band rows
    WLb, JLb = g["WLb"], g["JLb"]
    f32 = mybir.dt.float32
    bf16 = mybir.dt.bfloat16
    MULT = mybir.AluOpType.mult
    ADD = mybir.AluOpType.add
    SUB = mybir.AluOpType.subtract
    ACTF = mybir.ActivationFunctionType

    nc = bacc.Bacc("TRN2", target_bir_lowering=False, debug=False,
                   num_devices=NCORES)

    kmX = nc.dram_tensor("kmX", [NJOB, 128, KMW], bf16, kind="ExternalInput")
    smX = nc.dram_tensor("smX", [Nc + C24, 128, 2, 3, Ny], bf16,
                         kind="ExternalInput")
    ASd = nc.dram_tensor("ASd", [2, 5, 128, Ny], bf16, kind="ExternalInput")
    ASPMd = nc.dram_tensor("ASPMd", [5, 128, 128], bf16,
                           kind="ExternalInput")
    exd = nc.dram_tensor("exd", [g["ex_tot"]], bf16, kind="ExternalInput")
    eyd = nc.dram_tensor("eyd", [g["ey_tot"]], bf16, kind="ExternalInput")
    zzd = nc.dram_tensor("zzd", [1, 512], bf16, kind="ExternalInput")
    outp = nc.dram_tensor("outp", [NTX, 2, 128, Ny], bf16,
                          kind="ExternalOutput")

    from contextlib import ExitStack
    with tile.TileContext(nc) as tc, ExitStack() as ctx:
        const_pool = ctx.enter_context(tc.tile_pool(name="const", bufs=1))
        acc_pool = ctx.enter_context(tc.tile_pool(name="acc", bufs=1))
        aux_pool = ctx.enter_context(tc.tile_pool(name="aux", bufs=1))
        sm_pool = ctx.enter_context(tc.tile_pool(name="sm", bufs=1))
        sm24_pool = ctx.enter_context(tc.tile_pool(name="sm24", bufs=2))
        km_pool = ctx.enter_context(tc.tile_pool(name="km", bufs=2))
        t1_pool = ctx.enter_context(tc.tile_pool(name="t1", bufs=2))
        im_pool = ctx.enter_context(tc.tile_pool(name="im", bufs=2))
        scr_pool = ctx.enter_context(tc.tile_pool(name="scr", bufs=2))
        p_pool = ctx.enter_context(tc.tile_pool(name="prod", bufs=2))
        mid_pool = ctx.enter_context(tc.tile_pool(name="mid", bufs=2))
        imc_pool = ctx.enter_context(tc.tile_pool(name="imc", bufs=2))
        ex_pool = ctx.enter_context(tc.tile_pool(name="ex", bufs=2))
        ey_pool = ctx.enter_context(tc.tile_pool(name="ey", bufs=2))
        eyim_pool = ctx.enter_context(tc.tile_pool(name="eyim", bufs=2))
        psum_p1 = ctx.enter_context(tc.tile_pool(name="ps1", bufs=2,
                                                 space="PSUM"))
        psum_p2 = ctx.enter_context(tc.tile_pool(name="ps2", bufs=2,
                                                 space="PSUM"))
        psum_pw = ctx.enter_context(tc.tile_pool(name="psw", bufs=2,
                                                 space="PSUM"))

        # ---- constants ----
        AS = []
        for s in range(2):
            t = const_pool.tile([128, 5, Ny], bf16, tag=f"AS{s}")
            nc.sync.dma_start(t[:, :, :], ASd.ap()[s].transpose([1, 0, 2]))
            AS.append(t)
        ASPM = const_pool.tile([128, 5, 128], bf16, tag="ASPM")
        nc.sync.dma_start(ASPM[:, :, :], ASPMd.ap().transpose([1, 0, 2]))
        zz = const_pool.tile([1, 512], bf16, tag="zz")
        nc.sync.dma_start(zz[:, :], zzd.ap()[:, :])

        acc = {}
        for bx in range(NTX):
            for comp in range(2):
                t = acc_pool.tile([128, Ny], bf16, tag=f"acc{bx}_{comp}",
                                  name=f"acc{bx}_{comp}")
                acc[(bx, comp)] = t

        # ---- one FFT + coil-combine job ----
        def fft_job(job_id, sm_idx, auxt, first):
            km = km_pool.tile([128, KMW], bf16, tag="km", name=f"km{job_id}")
            nc.sync.dma_start(km[:, :], kmX.ap()[job_id])
            smt = get_sm(sm_idx)
            km5 = km[:, 0:1600].rearrange("p (c x) -> p c x", c=5, x=Ny)

            # pass 1: 4 plain chains + merged x-tail chain -> T1S [128,5,320]
            t1s = t1_pool.tile([128, 5, Ny], bf16, tag="t1s",
                               name=f"t1s{job_id}")
            for (c5, mlo, aset) in ((0, 0, 0), (1, 0, 1),
                                    (2, 128, 0), (3, 128, 1)):
                ps = psum_p1.tile([128, Ny], f32, tag="p1", name="p1")
                for c in range(5):
                    nc.tensor.matmul(ps[:, :], km5[:, c, mlo:mlo + 128],
                                     AS[aset][:, c, :],
                                     start=(c == 0), stop=(c == 4))
                nc.scalar.activation(t1s[:, c5, :], ps[:, :], ACTF.Copy)
            ps = psum_p1.tile([128, Ny], f32, tag="p1", name="p1m2")
            m2lhs = [
                km[:, 1920:2048],
                km[:, 1664:1792],
                km[:, 2048:2176],
                km[:, 1792:1920],
                km[:, 1536:1664],
            ]
            for c in range(5):
                nc.tensor.matmul(ps[:, :], m2lhs[c], AS[0][:, c, :],
                                 start=(c == 0), stop=(c == 4))
            nc.scalar.activation(t1s[:, 4, :], ps[:, :], ACTF.Copy)

            # pass 2: 4 full chains + merged x-tail chain -> im [128,2,3,320]
            imt = im_pool.tile([128, 2, 3, Ny], bf16, tag="im",
                               name=f"im{job_id}")
            for comp in range(2):
                for m in range(2):
                    ps = psum_p2.tile([128, Ny], f32, tag="p2", name="p2")
                    for c in range(5):
                        nc.tensor.matmul(
                            ps[:, :], AS[comp][:, c, 128 * m:128 * m + 128],
                            t1s[:, c, :], start=(c == 0), stop=(c == 4))
                    nc.scalar.activation(imt[:, comp, m, :], ps[:, :],
                                         ACTF.Copy)
            # merged tail: psum rows 0:64 = re x-tail, 64:128 = im x-tail
            ps = psum_p2.tile([128, Ny], f32, tag="p2", name="p2tail")
            for c in range(5):
                nc.tensor.matmul(ps[:, :], ASPM[:, c, :], t1s[:, c, :],
                                 start=(c == 0), stop=(c == 4))
            scr = scr_pool.tile([128, Ny], bf16, tag="scr",
                                name=f"scr{job_id}")
            nc.scalar.activation(scr[:, :], ps[:, :], ACTF.Copy)
            # rows 64:128 of the m2 planes are masked by zeroed smaps but
            # must hold finite values - cover them from scr too.
            nc.sync.dma_start(imt[:, 0, 2, :], scr[:, :])
            nc.sync.dma_start(imt[0:64, 1, 2, :], scr[64:128, :])
            nc.sync.dma_start(imt[64:128, 1, 2, :], scr[64:128, :])

            # coil combine: P = (sr*imr, si*imi, sr*imi, si*imr); real-part
            # accumulation on DVE, imag-part on Pool (engine balance)
            P = p_pool.tile([128, 4, 3, Ny], bf16, tag="P", name=f"P{job_id}")
            nc.vector.tensor_tensor(P[:, 0:2, :, :], smt[:, 0:2, :, :],
                                    imt[:, 0:2, :, :], MULT)
            nc.vector.tensor_tensor(P[:, 2:4, :, :], smt[:, 0:2, :, :],
                                    imt[:, 1::-1, :, :], MULT)
            if first:
                nc.vector.tensor_tensor(auxt[:, 0, :, :], P[:, 0, :, :],
                                        P[:, 1, :, :], ADD)
                nc.gpsimd.tensor_tensor(auxt[:, 1, :, :], P[:, 2, :, :],
                                        P[:, 3, :, :], SUB)
            else:
                nc.vector.tensor_tensor(auxt[:, 0, :, :], auxt[:, 0, :, :],
                                        P[:, 0, :, :], ADD)
                nc.vector.tensor_tensor(auxt[:, 0, :, :], auxt[:, 0, :, :],
                                        P[:, 1, :, :], ADD)
                nc.gpsimd.tensor_tensor(auxt[:, 1, :, :], auxt[:, 1, :, :],
                                        P[:, 2, :, :], ADD)
                nc.gpsimd.tensor_tensor(auxt[:, 1, :, :], auxt[:, 1, :, :],
                                        P[:, 3, :, :], SUB)

        sm_tiles = {}

        def get_sm(idx):
            if idx not in sm_tiles:
                pool = sm_pool if idx < Nc else sm24_pool
                t = pool.tile([128, 2, 3, Ny], bf16,
                              tag=("sm24" if idx >= Nc else f"sm{idx}"),
                              name=f"sm{idx}")
                nc.sync.dma_start(t[:, :, :, :], smX.ap()[idx])
                sm_tiles[idx] = t
            return sm_tiles[idx]

        # ---- warp chunks for one slot ----
        def warp_chunks(slot, auxt, first_acc):
            imc = imc_pool.tile([128, 2, NTX * NTY], bf16, tag="imc",
                                name=f"imc{slot}")

            def prep():
                for comp in range(2):
                    mid = mid_pool.tile([128, 3, 4, NTY], bf16, tag="mid",
                                        name=f"mid{slot}_{comp}")
                    for k in range(3):
                        nc.scalar.activation(
                            mid[:, k, :, :],
                            auxt[:, comp, k, :].rearrange(
                                "p (g ul) -> p g ul", g=NTY, ul=4)
                            .transpose([0, 2, 1]),
                            ACTF.Copy)
                    with nc.allow_non_contiguous_dma(reason="imc gather"):
                        for a in range(NTX):
                            k, a4 = a // 4, a % 4
                            nc.sync.dma_start(
                                imc[:, comp, NTY * a:NTY * a + NTY],
                                mid[32 * a4:32 * a4 + 32, k, :, :]
                                .rearrange("p ul g -> p (ul g)"))

            def mk_bx(bx):
                WL = int(WLb[slot, bx])
                JLc = int(JLb[slot, bx])

                def chunk():
                    ext = ex_pool.tile([128, NTY, WL], bf16, tag="ex",
                                       name=f"ex{slot}_{bx}")
                    nc.sync.dma_start(
                        ext[:, :, :],
                        exd.ap()[int(g["ex_off"][slot, bx]):
                                 int(g["ex_off"][slot, bx]) +
                                 128 * NTY * WL]
                        .rearrange("(p t w) -> p t w", p=128, t=NTY, w=WL))
                    eyt = ey_pool.tile([128, JLc, NTY], bf16, tag="ey",
                                       name=f"ey{slot}_{bx}")
                    nc.sync.dma_start(
                        eyt[:, :, :],
                        eyd.ap()[int(g["ey_off"][slot, bx]):
                                 int(g["ey_off"][slot, bx]) +
                                 128 * JLc * NTY]
                        .rearrange("(p j t) -> p j t", p=128, j=JLc, t=NTY))
                    eyim = eyim_pool.tile([128, 2, JLc, NTY], bf16,
                                          tag="eyim",
                                          name=f"eyim{slot}_{bx}")
                    nc.vector.tensor_tensor(
                        eyim[:, :, :, :],
                        eyt.unsqueeze(1).broadcast_to([128, 2, JLc, NTY]),
                        imc[:, :, NTY * bx:NTY * bx + NTY].unsqueeze(2)
                        .broadcast_to([128, 2, JLc, NTY]),
                        MULT)
                    pw = []
                    for comp in range(2):
                        p = psum_pw.tile([128, PWW], f32, tag=f"pw{comp}",
                                         name=f"pw{slot}_{bx}_{comp}")
                        nc.tensor.matmul(p[:, :], zz[0:1, 0:128],
                                         zz[0:1, 0:PWW], start=True,
                                         stop=False, skip_group_check=True)
                        pw.append(p)
                    for ti in range(NTY):
                        obr = int(ox[slot, bx, ti])
                        obc = int(oy[slot, bx, ti])
                        for comp in range(2):
                            nc.tensor.matmul(
                                pw[comp][obr:obr + WL, obc:obc + JLc],
                                ext[:, ti, :], eyim[:, comp, :, ti],
                                start=False, stop=(ti == NTY - 1),
                                skip_group_check=True)
                    for comp in range(2):
                        if first_acc:
                            nc.scalar.activation(
                                acc[(bx, comp)][:, :],
                                pw[comp][:, Dy:Dy + Ny], ACTF.Copy)
                        else:
                            nc.vector.tensor_tensor(
                                acc[(bx, comp)][:, :], acc[(bx, comp)][:, :],
                                pw[comp][:, Dy:Dy + Ny], ADD)
                return chunk

            return [prep] + [mk_bx(bx) for bx in range(NTX)]

        # ---- schedule (slot-major) ----
        aux = {}
        for slot in range(NSLOT):
            aux[slot] = aux_pool.tile([128, 2, 3, Ny], bf16,
                                      tag=f"aux{slot}", name=f"aux{slot}")

        job = 0
        # phase A: slot 0, no warps yet
        for c in range(Nc):
            fft_job(job, c, aux[0], first=(c == 0))
            job += 1
        # phase B: slot 1 || warp of slot 0
        pend = warp_chunks(0, aux[0], first_acc=True)
        pi = 0
        for c in range(Nc):
            fft_job(job, c, aux[1], first=(c == 0))
            job += 1
            goal = ((c + 1) * len(pend) + Nc - 1) // Nc
            while pi < min(goal, len(pend)):
                pend[pi]()
                pi += 1
        while pi < len(pend):
            pend[pi]()
            pi += 1
        # phase C+D: f24 jobs then slot 2 || warps of slots 1 and 3
        pend = warp_chunks(1, aux[1], first_acc=False) + \
            warp_chunks(3, aux[3], first_acc=False)
        pi = 0
        jobs_cd = [(Nc + c, 3, c == 0) for c in range(C24)] + \
            [(c, 2, c == 0) for c in range(Nc)]
        for j, (smi, sl, first) in enumerate(jobs_cd):
            fft_job(job, smi, aux[sl], first)
            job += 1
            goal = ((j + 1) * len(pend) + len(jobs_cd) - 1) // len(jobs_cd)
            while pi < min(goal, len(pend)):
                pend[pi]()
                pi += 1
        while pi < len(pend):
            pend[pi]()
            pi += 1
        assert job == NJOB
        for ch in warp_chunks(2, aux[2], first_acc=False):  # tail warp
            ch()

        for bx in range(NTX):
            for comp in range(2):
                nc.sync.dma_start(outp.ap()[bx, comp],
                                  acc[(bx, comp)][:, :])

    nc.compile()
    return nc


def _host_prep(kspace_r, kspace_i, mask, smaps_r, smaps_i, flow, g):
    import ml_dtypes
    bf16 = ml_dtypes.bfloat16
    f64 = np.float64
    Dx, Dy = g["Dx"], g["Dy"]
    sx, sy, WLb, JLb = g["sx"], g["sy"], g["WLb"], g["JLb"]

    Ar, Ai = _build_A()
    ASr = _stack5(Ar, -Ai)
    ASi = _stack5(Ai, Ar)
    ASd = np.stack([ASr, ASi]).astype(bf16)
    ASPMd = np.zeros((5, 128, 128), f64)
    ASPMd[:, :, 0:64] = ASr[:, :, 256:320]
    ASPMd[:, :, 64:128] = ASi[:, :, 256:320]
    ASPMd = ASPMd.astype(bf16)

    ksr = kspace_r.astype(f64)
    ksi = kspace_i.astype(f64)
    maskf = mask.astype(f64)

    def jobs_for_core(r):
        fr = [FR_FULL * r + s for s in range(FR_FULL)]
        out = [(c, fr[0]) for c in range(Nc)]
        out += [(c, fr[1]) for c in range(Nc)]
        out += [(C24 * r + c, Nt - 1) for c in range(C24)]
        out += [(c, fr[2]) for c in range(Nc)]
        return out

    # smaps [128, 2 planes (sr, si), 3 m, 320]; m2 rows 64:128 zeroed
    smT = np.zeros((Nc, 128, 2, 3, Ny), f64)
    for c in range(Nc):
        for m in range(3):
            rows = min(128, Nx - 128 * m)
            smT[c, 0:rows, 0, m, :] = smaps_r[128 * m:128 * m + rows, :, c]
            smT[c, 0:rows, 1, m, :] = smaps_i[128 * m:128 * m + rows, :, c]
    smT[:, 64:128, :, 2, :] = 0.0

    # hat tables per frame, packed flat with per-(slot,bx) shapes
    X, Y = np.meshgrid(np.arange(Nx, dtype=f64), np.arange(Ny, dtype=f64),
                       indexing="ij")
    ex_all = {}
    ey_all = {}
    for t in range(Nt):
        slot_t = FR_FULL if t == Nt - 1 else t % FR_FULL
        px = np.clip(X + flow[:, :, 0, t].astype(f64), 0.0, Nx - 1.0)
        py = np.clip(Y + flow[:, :, 1, t].astype(f64), 0.0, Ny - 1.0)
        pxt = px.reshape(NTX, BX, NTY, BY)   # [bx, vl, by, ul]
        pyt = py.reshape(NTX, BX, NTY, BY)
        ex_bx = []
        ey_bx = []
        for bx in range(NTX):
            WL = int(WLb[slot_t, bx])
            JLc = int(JLb[slot_t, bx])
            pxq = pxt[bx].transpose(0, 2, 1).reshape(128, NTY)  # q=4vl+ul
            pyq = pyt[bx].transpose(0, 2, 1).reshape(128, NTY)
            xrow = sx[slot_t, bx].astype(f64)[None, :, None] + \
                np.arange(WL, dtype=f64)[None, None, :]
            ex_bx.append(np.maximum(
                0.0, 1.0 - np.abs(xrow - pxq[:, :, None])
            ).astype(bf16))                                   # [128,NTY,WL]
            yrow = sy[slot_t, bx].astype(f64)[None, :, None] + \
                np.arange(JLc, dtype=f64)[None, None, :]
            eyv = np.maximum(0.0, 1.0 - np.abs(yrow - pyq[:, :, None]))
            ey_bx.append(eyv.astype(bf16).transpose(0, 2, 1))  # [128,JL,NTY]
        ex_all[t] = ex_bx
        ey_all[t] = ey_bx

    zz = np.zeros((1, 512), bf16)

    in_maps = []
    for r in range(NCORES):
        jl = jobs_for_core(r)
        kmX = np.zeros((NJOB, 128, KMW), bf16)
        for ji, (c, t) in enumerate(jl):
            kmr = (ksr[:, :, c] * maskf[:, :, c, t]).T   # [y, x]
            kmi = (ksi[:, :, c] * maskf[:, :, c, t]).T
            kmX[ji] = _km_pack(kmr, kmi).astype(bf16)
        smXc = np.zeros((Nc + C24, 128, 2, 3, Ny), f64)
        smXc[0:Nc] = smT
        for c in range(C24):
            smXc[Nc + c] = smT[C24 * r + c]
        slot_frames = [FR_FULL * r, FR_FULL * r + 1, FR_FULL * r + 2, Nt - 1]
        exd = np.zeros(g["ex_tot"], bf16)
        eyd = np.zeros(g["ey_tot"], bf16)
        for s in range(NSLOT):
            t = slot_frames[s]
            for bx in range(NTX):
                o = int(g["ex_off"][s, bx])
                exd[o:o + ex_all[t][bx].size] = ex_all[t][bx].ravel()
                o = int(g["ey_off"][s, bx])
                eyd[o:o + ey_all[t][bx].size] = ey_all[t][bx].ravel()
        in_maps.append({
            "kmX": kmX,
            "smX": smXc.astype(bf16),
            "ASd": ASd,
            "ASPMd": ASPMd,
            "exd": exd,
            "eyd": eyd,
            "zzd": zz,
        })
    return in_maps


def kernel(kspace_r, kspace_i, mask, smaps_r, smaps_i, flow):
    from concourse.bass_utils import run_bass_kernel_spmd

    flow = np.asarray(flow, np.float32)
    g = _geometry(flow)
    Dx = g["Dx"]

    key = (g["Dx"], g["Dy"], g["sx"].tobytes(), g["sy"].tobytes(),
           g["WLb"].tobytes(), g["JLb"].tobytes())
    if key not in _CACHE:
        _CACHE[key] = _build_program(g)
    nc = _CACHE[key]

    in_maps = _host_prep(
        np.asarray(kspace_r, np.float32), np.asarray(kspace_i, np.float32),
        np.asarray(mask, np.float32), np.asarray(smaps_r, np.float32),
        np.asarray(smaps_i, np.float32), flow, g)

    res = run_bass_kernel_spmd(nc, in_maps, core_ids=list(range(NCORES)))

    out = np.zeros((2, Nx, Ny), np.float64)
    for r in range(NCORES):
        o = res.results[r]["outp"].astype(np.float64)   # [NTX, 2, 128, 320]
        for bx in range(NTX):
            x0 = 32 * bx - Dx
            qlo = max(0, -x0)
            qhi = min(128, Nx - x0, 32 + 2 * Dx)
            for comp in range(2):
                out[comp, x0 + qlo:x0 + qhi, :] += o[bx, comp, qlo:qhi, :]
    return np.stack([out[0], out[1]], axis=-1).astype(np.float32)


# revision 37
# speedup vs baseline: 1.0307x; 1.0109x over previous
# Trainium2 Bass kernel for nn_BatchelorAdj (motion-compensated MRI recon
# adjoint):  out = sum_t W_t^T( sum_c conj(S_c) . IFFT2c(K_c . M_ct) )
#
# v4 design (cost-model driven):
#  - host precomputes km = kspace*mask per (coil,frame) in a stacked-chunk
#    bf16 layout so each IFFT matmul pass runs 5-instruction chains over a
#    640-long stacked contraction ([kmr;kmi] x [A_r;-A_i]); pass 1 merges
#    the 64-row x-tail of the r/i outputs into one 128-row chain, and pass
#    2 merges the two comps' x-tail chains via a stacked ASPM constant
#    (psum rows 0:64 = re tail, 64:128 = im tail) + partition-shift DMA.
#  - smaps: 2 planes (sr, si) resident in SBUF for all 16 coils (+2 f24
#    slots); the second coil-combine product pair uses a negative-stride
#    plane view of the image tile, so no duplicated sr plane is shipped.
#  - adjoint warp: host ships bf16 bilinear hat tables with per-(slot,bx)
#    widths and per-(slot,bx,ti) start offsets that are the union of the 8
#    cores' local windows (shared program => shared offsets), packed flat
#    in DRAM for full-rate DMA.
#  - schedule is slot-major: slot0 jobs; slot1 jobs || warp0; f24 jobs +
#    slot2 jobs || warp1+warp3; tail warp2. Warp table DMA spreads over
#    the whole fft timeline.
#  - per-bx psum bands [128, 324+2D] are flushed into per-bx bf16 SBUF
#    accumulators (Act copy on the first slot, DVE add after); host
#    applies x-shifts in the final cross-core reduce.
import math
import numpy as np

Nx = Ny = 320
Nc = 16
Nt = 25
NCORES = 8
BX, BY = 32, 4
NTX, NTY = Nx // BX, Ny // BY       # 10, 80
FR_FULL = 3
C24 = Nc // NCORES                  # 2
NSLOT = FR_FULL + 1                 # 4
NJOB = FR_FULL * Nc + C24           # 50 fft jobs per core
KMW = 2176                          # km tile width (see _km_pack)

_CACHE = {}


def _build_A():
    j = np.arange(Nx)
    F = np.exp(2j * np.pi * np.outer(j, j) / Nx) / np.sqrt(Nx)
    P = np.zeros((Nx, Nx))
    P[j, (j + Nx // 2) % Nx] = 1.0
    A = P @ F @ P
    return A.real.astype(np.float64), A.imag.astype(np.float64)


def _stack5(M, N):
    """Chunk the 640-row stack [M;N] of two 320-row mats into [5,128,cols]:
    (M0, N0, M1, N1, [M2;N2])."""
    out = np.zeros((5, 128, M.shape[1]), np.float64)
    out[0] = M[0:128]
    out[1] = N[0:128]
    out[2] = M[128:256]
    out[3] = N[128:256]
    out[4, 0:64] = M[256:320]
    out[4, 64:128] = N[256:320]
    return out


def _km_pack(kmr, kmi):
    """km [y,x] pair -> [128, 2176] layout.
    cols [0:1600]:   km5 chunks (kmr0,kmi0,kmr1,kmi1,[kmr2;kmi2]) full x.
    cols [1600:1664]: [kmi2 ; -kmr2] @ x 256:320 (partition-stacked)
    cols [1664:1920]: (kmi-y0, -kmr-y0, kmi-y1, -kmr-y1) @ x 256:320
    cols [1920:2176]: (kmr-y0, kmi-y0, kmr-y1, kmi-y1) @ x 256:320."""
    out = np.zeros((128, KMW), np.float64)
    km5 = _stack5(kmr, kmi)
    out[:, 0:1600] = km5.transpose(1, 0, 2).reshape(128, 1600)
    out[0:64, 1600:1664] = kmi[256:320, 256:320]
    out[64:128, 1600:1664] = -kmr[256:320, 256:320]
    out[:, 1664:1728] = kmi[0:128, 256:320]
    out[:, 1728:1792] = -kmr[0:128, 256:320]
    out[:, 1792:1856] = kmi[128:256, 256:320]
    out[:, 1856:1920] = -kmr[128:256, 256:320]
    out[:, 1920:1984] = kmr[0:128, 256:320]
    out[:, 1984:2048] = kmi[0:128, 256:320]
    out[:, 2048:2112] = kmr[128:256, 256:320]
    out[:, 2112:2176] = kmi[128:256, 256:320]
    return out


def _geometry(flow):
    """Union-local warp windows shared by all 8 cores.

    Returns dict with per-(slot,bx,ti) y/x window starts (sy, sx), per-
    (slot,bx) widths (JLb, WLb), flat ex/ey table offsets, and Dx/Dy."""
    f64 = np.float64
    Dx = int(math.ceil(np.abs(flow[:, :, 0, :]).max()))
    Dy = int(math.ceil(np.abs(flow[:, :, 1, :]).max()))
    W = 32 + 2 * Dx
    Xg, Yg = np.meshgrid(np.arange(Nx, dtype=f64), np.arange(Ny, dtype=f64),
                         indexing="ij")
    fminx = np.zeros((Nt, NTX, NTY), np.int64)
    fmaxx = np.zeros((Nt, NTX, NTY), np.int64)
    fminy = np.zeros((Nt, NTX, NTY), np.int64)
    fmaxy = np.zeros((Nt, NTX, NTY), np.int64)
    for t in range(Nt):
        px = np.clip(Xg + flow[:, :, 0, t].astype(f64), 0.0, Nx - 1.0)
        py = np.clip(Yg + flow[:, :, 1, t].astype(f64), 0.0, Ny - 1.0)
        fx = np.floor(px).astype(np.int64).reshape(NTX, BX, NTY, BY)
        fy = np.floor(py).astype(np.int64).reshape(NTX, BX, NTY, BY)
        fminx[t] = fx.min(axis=(1, 3))
        fmaxx[t] = fx.max(axis=(1, 3))
        fminy[t] = fy.min(axis=(1, 3))
        fmaxy[t] = fy.max(axis=(1, 3))

    sx = np.zeros((NSLOT, NTX, NTY), np.int64)
    ex_hi = np.zeros((NSLOT, NTX, NTY), np.int64)
    sy = np.zeros((NSLOT, NTX, NTY), np.int64)
    ey_hi = np.zeros((NSLOT, NTX, NTY), np.int64)
    for s in range(FR_FULL):
        frames = [FR_FULL * r + s for r in range(NCORES)]
        sx[s] = fminx[frames].min(axis=0)
        ex_hi[s] = fmaxx[frames].max(axis=0)
        sy[s] = fminy[frames].min(axis=0)
        ey_hi[s] = fmaxy[frames].max(axis=0)
    sx[FR_FULL], ex_hi[FR_FULL] = fminx[Nt - 1], fmaxx[Nt - 1]
    sy[FR_FULL], ey_hi[FR_FULL] = fminy[Nt - 1], fmaxy[Nt - 1]

    # x windows: PE psum writes need base partition in {0,32,64}, so pin
    # the window start at the band start and trim only the upper edge.
    bxlo = (32 * np.arange(NTX) - Dx)[None, :, None]
    sx = np.broadcast_to(bxlo, sx.shape).copy()
    WLb = np.minimum((ex_hi - sx + 2).max(axis=2), W)   # [NSLOT, NTX]
    JLb = (ey_hi - sy + 2).max(axis=2)
    sy = np.minimum(sy, Ny + Dy + 4 - JLb[:, :, None])

    ex_off = np.zeros((NSLOT, NTX), np.int64)
    ey_off = np.zeros((NSLOT, NTX), np.int64)
    pos = 0
    for s in range(NSLOT):
        for bx in range(NTX):
            ex_off[s, bx] = pos
            pos += 128 * NTY * int(WLb[s, bx])
    ex_tot = pos
    pos = 0
    for s in range(NSLOT):
        for bx in range(NTX):
            ey_off[s, bx] = pos
            pos += 128 * int(JLb[s, bx]) * NTY
    ey_tot = pos
    return dict(Dx=Dx, Dy=Dy, W=W, sx=sx, sy=sy, WLb=WLb, JLb=JLb,
                ex_off=ex_off, ey_off=ey_off, ex_tot=ex_tot, ey_tot=ey_tot)


def _build_program(g):
    from concourse import bass, bacc, tile, mybir

    Dx, Dy, W = g["Dx"], g["Dy"], g["W"]
    PWW = Ny + 2 * Dy + 4              # psum band columns (y + Dy offset)
    oy = g["sy"] + Dy                  # psum band col starts
    ox = g["sx"] - (32 * np.arange(NTX) - Dx)[None, :, None]  # band rows
    WLb, JLb = g["WLb"], g["JLb"]
    f32 = mybir.dt.float32
    bf16 = mybir.dt.bfloat16
    MULT = mybir.AluOpType.mult
    ADD = mybir.AluOpType.add
    SUB = mybir.AluOpType.subtract
    ACTF = mybir.ActivationFunctionType

    nc = bacc.Bacc("TRN2", target_bir_lowering=False, debug=False,
                   num_devices=NCORES)

    kmX = nc.dram_tensor("kmX", [NJOB, 128, KMW], bf16, kind="ExternalInput")
    smX = nc.dram_tensor("smX", [Nc + C24, 128, 2, 3, Ny], bf16,
                         kind="ExternalInput")
    ASd = nc.dram_tensor("ASd", [2, 5, 128, Ny], bf16, kind="ExternalInput")
    ASPMd = nc.dram_tensor("ASPMd", [5, 128, 128], bf16,
                           kind="ExternalInput")
    exd = nc.dram_tensor("exd", [g["ex_tot"]], bf16, kind="ExternalInput")
    eyd = nc.dram_tensor("eyd", [g["ey_tot"]], bf16, kind="ExternalInput")
    zzd = nc.dram_tensor("zzd", [1, 512], bf16, kind="ExternalInput")
    outp = nc.dram_tensor("outp", [NTX, 2, 128, Ny], bf16,
                          kind="ExternalOutput")

    from contextlib import ExitStack
    with tile.TileContext(nc) as tc, ExitStack() as ctx:
        const_pool = ctx.enter_context(tc.tile_pool(name="const", bufs=1))
        acc_pool = ctx.enter_context(tc.tile_pool(name="acc", bufs=1))
        aux_pool = ctx.enter_context(tc.tile_pool(name="aux", bufs=1))
        sm_pool = ctx.enter_context(tc.tile_pool(name="sm", bufs=1))
        sm24_pool = ctx.enter_context(tc.tile_pool(name="sm24", bufs=2))
        km_pool = ctx.enter_context(tc.tile_pool(name="km", bufs=2))
        t1_pool = ctx.enter_context(tc.tile_pool(name="t1", bufs=2))
        im_pool = ctx.enter_context(tc.tile_pool(name="im", bufs=2))
        scr_pool = ctx.enter_context(tc.tile_pool(name="scr", bufs=2))
        p_pool = ctx.enter_context(tc.tile_pool(name="prod", bufs=2))
        mid_pool = ctx.enter_context(tc.tile_pool(name="mid", bufs=2))
        imc_pool = ctx.enter_context(tc.tile_pool(name="imc", bufs=2))
        ex_pool = ctx.enter_context(tc.tile_pool(name="ex", bufs=2))
        ey_pool = ctx.enter_context(tc.tile_pool(name="ey", bufs=2))
        eyim_pool = ctx.enter_context(tc.tile_pool(name="eyim", bufs=2))
        psum_p1 = ctx.enter_context(tc.tile_pool(name="ps1", bufs=2,
                                                 space="PSUM"))
        psum_p2 = ctx.enter_context(tc.tile_pool(name="ps2", bufs=2,
                                                 space="PSUM"))
        psum_pw = ctx.enter_context(tc.tile_pool(name="psw", bufs=2,
                                                 space="PSUM"))

        # ---- constants ----
        AS = []
        for s in range(2):
            t = const_pool.tile([128, 5, Ny], bf16, tag=f"AS{s}")
            nc.sync.dma_start(t[:, :, :], ASd.ap()[s].transpose([1, 0, 2]))
            AS.append(t)
        ASPM = const_pool.tile([128, 5, 128], bf16, tag="ASPM")
        nc.sync.dma_start(ASPM[:, :, :], ASPMd.ap().transpose([1, 0, 2]))
        zz = const_pool.tile([1, 512], bf16, tag="zz")
        nc.sync.dma_start(zz[:, :], zzd.ap()[:, :])

        acc = {}
        for bx in range(NTX):
            for comp in range(2):
                t = acc_pool.tile([128, Ny], bf16, tag=f"acc{bx}_{comp}",
                                  name=f"acc{bx}_{comp}")
                acc[(bx, comp)] = t

        # ---- one FFT + coil-combine job ----
        def fft_job(job_id, sm_idx, auxt, first):
            km = km_pool.tile([128, KMW], bf16, tag="km", name=f"km{job_id}")
            nc.sync.dma_start(km[:, :], kmX.ap()[job_id])
            smt = get_sm(sm_idx)
            km5 = km[:, 0:1600].rearrange("p (c x) -> p c x", c=5, x=Ny)

            # pass 1: 4 plain chains + merged x-tail chain -> T1S [128,5,320]
            t1s = t1_pool.tile([128, 5, Ny], bf16, tag="t1s",
                               name=f"t1s{job_id}")
            for (c5, mlo, aset) in ((0, 0, 0), (1, 0, 1),
                                    (2, 128, 0), (3, 128, 1)):
                ps = psum_p1.tile([128, Ny], f32, tag="p1", name="p1")
                for c in range(5):
                    nc.tensor.matmul(ps[:, :], km5[:, c, mlo:mlo + 128],
                                     AS[aset][:, c, :],
                                     start=(c == 0), stop=(c == 4))
                nc.scalar.activation(t1s[:, c5, :], ps[:, :], ACTF.Copy)
            ps = psum_p1.tile([128, Ny], f32, tag="p1", name="p1m2")
            m2lhs = [
                km[:, 1920:2048],
                km[:, 1664:1792],
                km[:, 2048:2176],
                km[:, 1792:1920],
                km[:, 1536:1664],
            ]
            for c in range(5):
                nc.tensor.matmul(ps[:, :], m2lhs[c], AS[0][:, c, :],
                                 start=(c == 0), stop=(c == 4))
            nc.scalar.activation(t1s[:, 4, :], ps[:, :], ACTF.Copy)

            # pass 2: 4 full chains + merged x-tail chain -> im [128,2,3,320]
            imt = im_pool.tile([128, 2, 3, Ny], bf16, tag="im",
                               name=f"im{job_id}")
            for comp in range(2):
                for m in range(2):
                    ps = psum_p2.tile([128, Ny], f32, tag="p2", name="p2")
                    for c in range(5):
                        nc.tensor.matmul(
                            ps[:, :], AS[comp][:, c, 128 * m:128 * m + 128],
                            t1s[:, c, :], start=(c == 0), stop=(c == 4))
                    nc.scalar.activation(imt[:, comp, m, :], ps[:, :],
                                         ACTF.Copy)
            # merged tail: psum rows 0:64 = re x-tail, 64:128 = im x-tail
            ps = psum_p2.tile([128, Ny], f32, tag="p2", name="p2tail")
            for c in range(5):
                nc.tensor.matmul(ps[:, :], ASPM[:, c, :], t1s[:, c, :],
                                 start=(c == 0), stop=(c == 4))
            scr = scr_pool.tile([128, Ny], bf16, tag="scr",
                                name=f"scr{job_id}")
            nc.scalar.activation(scr[:, :], ps[:, :], ACTF.Copy)
            # rows 64:128 of the m2 planes are masked by zeroed smaps but
            # must hold finite values - cover them from scr too.
            nc.sync.dma_start(imt[:, 0, 2, :], scr[:, :])
            nc.sync.dma_start(imt[0:64, 1, 2, :], scr[64:128, :])
            nc.sync.dma_start(imt[64:128, 1, 2, :], scr[64:128, :])

            # coil combine: P = (sr*imr, si*imi, sr*imi, si*imr); real-part
            # accumulation on DVE, imag-part on Pool (engine balance)
            P = p_pool.tile([128, 4, 3, Ny], bf16, tag="P", name=f"P{job_id}")
            nc.vector.tensor_tensor(P[:, 0:2, :, :], smt[:, 0:2, :, :],
                                    imt[:, 0:2, :, :], MULT)
            nc.vector.tensor_tensor(P[:, 2:4, :, :], smt[:, 0:2, :, :],
                                    imt[:, 1::-1, :, :], MULT)
            if first:
                nc.vector.tensor_tensor(auxt[:, 0, :, :], P[:, 0, :, :],
                                        P[:, 1, :, :], ADD)
                nc.gpsimd.tensor_tensor(auxt[:, 1, :, :], P[:, 2, :, :],
                                        P[:, 3, :, :], SUB)
            else:
                nc.vector.tensor_tensor(auxt[:, 0, :, :], auxt[:, 0, :, :],
                                        P[:, 0, :, :], ADD)
                nc.vector.tensor_tensor(auxt[:, 0, :, :], auxt[:, 0, :, :],
                                        P[:, 1, :, :], ADD)
                nc.gpsimd.tensor_tensor(auxt[:, 1, :, :], auxt[:, 1, :, :],
                                        P[:, 2, :, :], ADD)
                nc.gpsimd.tensor_tensor(auxt[:, 1, :, :], auxt[:, 1, :, :],
                                        P[:, 3, :, :], SUB)

        sm_tiles = {}

        def get_sm(idx):
            if idx not in sm_tiles:
                pool = sm_pool if idx < Nc else sm24_pool
                t = pool.tile([128, 2, 3, Ny], bf16,
                              tag=("sm24" if idx >= Nc else f"sm{idx}"),
                              name=f"sm{idx}")
                nc.sync.dma_start(t[:, :, :, :], smX.ap()[idx])
                sm_tiles[idx] = t
            return sm_tiles[idx]

        # ---- warp chunks for one slot ----
        def warp_chunks(slot, auxt, first_acc):
            imc = imc_pool.tile([128, 2, NTX * NTY], bf16, tag="imc",
                                name=f"imc{slot}")

            def prep():
                for comp in range(2):
                    mid = mid_pool.tile([128, 3, 4, NTY], bf16, tag="mid",
                                        name=f"mid{slot}_{comp}")
                    for k in range(3):
                        nc.scalar.activation(
                            mid[:, k, :, :],
                            auxt[:, comp, k, :].rearrange(
                                "p (g ul) -> p g ul", g=NTY, ul=4)
                            .transpose([0, 2, 1]),
                            ACTF.Copy)
                    with nc.allow_non_contiguous_dma(reason="imc gather"):
                        for a in range(NTX):
                            k, a4 = a // 4, a % 4
                            nc.sync.dma_start(
                                imc[:, comp, NTY * a:NTY * a + NTY],
                                mid[32 * a4:32 * a4 + 32, k, :, :]
                                .rearrange("p ul g -> p (ul g)"))

            def mk_bx(bx):
                WL = int(WLb[slot, bx])
                JLc = int(JLb[slot, bx])

                def chunk():
                    ext = ex_pool.tile([128, NTY, WL], bf16, tag="ex",
                                       name=f"ex{slot}_{bx}")
                    nc.sync.dma_start(
                        ext[:, :, :],
                        exd.ap()[int(g["ex_off"][slot, bx]):
                                 int(g["ex_off"][slot, bx]) +
                                 128 * NTY * WL]
                        .rearrange("(p t w) -> p t w", p=128, t=NTY, w=WL))
                    eyt = ey_pool.tile([128, JLc, NTY], bf16, tag="ey",
                                       name=f"ey{slot}_{bx}")
                    nc.sync.dma_start(
                        eyt[:, :, :],
                        eyd.ap()[int(g["ey_off"][slot, bx]):
                                 int(g["ey_off"][slot, bx]) +
                                 128 * JLc * NTY]
                        .rearrange("(p j t) -> p j t", p=128, j=JLc, t=NTY))
                    eyim = eyim_pool.tile([128, 2, JLc, NTY], bf16,
                                          tag="eyim",
                                          name=f"eyim{slot}_{bx}")
                    nc.vector.tensor_tensor(
                        eyim[:, :, :, :],
                        eyt.unsqueeze(1).broadcast_to([128, 2, JLc, NTY]),
                        imc[:, :, NTY * bx:NTY * bx + NTY].unsqueeze(2)
                        .broadcast_to([128, 2, JLc, NTY]),
                        MULT)
                    pw = []
                    for comp in range(2):
                        p = psum_pw.tile([128, PWW], f32, tag=f"pw{comp}",
                                         name=f"pw{slot}_{bx}_{comp}")
                        nc.tensor.matmul(p[:, :], zz[0:1, 0:128],
                                         zz[0:1, 0:PWW], start=True,
                                         stop=False, skip_group_check=True)
                        pw.append(p)
                    for ti in range(NTY):
                        obr = int(ox[slot, bx, ti])
                        obc = int(oy[slot, bx, ti])
                        for comp in range(2):
                            nc.tensor.matmul(
                                pw[comp][obr:obr + WL, obc:obc + JLc],
                                ext[:, ti, :], eyim[:, comp, :, ti],
                                start=False, stop=(ti == NTY - 1),
                                skip_group_check=True)
                    for comp in range(2):
                        if first_acc:
                            nc.scalar.activation(
                                acc[(bx, comp)][:, :],
                                pw[comp][:, Dy:Dy + Ny], ACTF.Copy)
                        else:
                            nc.vector.tensor_tensor(
                                acc[(bx, comp)][:, :], acc[(bx, comp)][:, :],
                                pw[comp][:, Dy:Dy + Ny], ADD)
                return chunk

            return [prep] + [mk_bx(bx) for bx in range(NTX)]

        # ---- schedule (slot-major) ----
        aux = {}
        for slot in range(NSLOT):
            aux[slot] = aux_pool.tile([128, 2, 3, Ny], bf16,
                                      tag=f"aux{slot}", name=f"aux{slot}")

        def run_phase(jobs, pend):
            nonlocal job
            pi = 0
            for j, (smi, sl, first) in enumerate(jobs):
                fft_job(job, smi, aux[sl], first)
                job += 1
                goal = ((j + 1) * len(pend) + len(jobs) - 1) // len(jobs)
                while pi < min(goal, len(pend)):
                    pend[pi]()
                    pi += 1
            while pi < len(pend):
                pend[pi]()
                pi += 1

        job = 0
        # phase A: slot 0 + f24 jobs, no warps yet
        run_phase([(c, 0, c == 0) for c in range(Nc)] +
                  [(Nc + c, 3, c == 0) for c in range(C24)], [])
        # warp3 is ready after phase A; split its chunks across B and C
        w3 = warp_chunks(3, aux[3], first_acc=False)
        # phase B: slot 1 || warp0 + first part of warp3
        run_phase([(c, 1, c == 0) for c in range(Nc)],
                  warp_chunks(0, aux[0], first_acc=True) + w3[:6])
        # phase C: slot 2 || rest of warp3 + warp1
        w1 = warp_chunks(1, aux[1], first_acc=False)
        run_phase([(c, 2, c == 0) for c in range(Nc)],
                  [w1[0]] + w3[6:] + w1[1:])
        assert job == NJOB
        for ch in warp_chunks(2, aux[2], first_acc=False):  # tail warp
            ch()

        for bx in range(NTX):
            for comp in range(2):
                nc.sync.dma_start(outp.ap()[bx, comp],
                                  acc[(bx, comp)][:, :])

    nc.compile()
    return nc


def _host_prep(kspace_r, kspace_i, mask, smaps_r, smaps_i, flow, g):
    import ml_dtypes
    bf16 = ml_dtypes.bfloat16
    f64 = np.float64
    Dx, Dy = g["Dx"], g["Dy"]
    sx, sy, WLb, JLb = g["sx"], g["sy"], g["WLb"], g["JLb"]

    Ar, Ai = _build_A()
    ASr = _stack5(Ar, -Ai)
    ASi = _stack5(Ai, Ar)
    ASd = np.stack([ASr, ASi]).astype(bf16)
    ASPMd = np.zeros((5, 128, 128), f64)
    ASPMd[:, :, 0:64] = ASr[:, :, 256:320]
    ASPMd[:, :, 64:128] = ASi[:, :, 256:320]
    ASPMd = ASPMd.astype(bf16)

    ksr = kspace_r.astype(f64)
    ksi = kspace_i.astype(f64)
    maskf = mask.astype(f64)

    def jobs_for_core(r):
        fr = [FR_FULL * r + s for s in range(FR_FULL)]
        out = [(c, fr[0]) for c in range(Nc)]
        out += [(C24 * r + c, Nt - 1) for c in range(C24)]
        out += [(c, fr[1]) for c in range(Nc)]
        out += [(c, fr[2]) for c in range(Nc)]
        return out

    # smaps [128, 2 planes (sr, si), 3 m, 320]; m2 rows 64:128 zeroed
    smT = np.zeros((Nc, 128, 2, 3, Ny), f64)
    for c in range(Nc):
        for m in range(3):
            rows = min(128, Nx - 128 * m)
            smT[c, 0:rows, 0, m, :] = smaps_r[128 * m:128 * m + rows, :, c]
            smT[c, 0:rows, 1, m, :] = smaps_i[128 * m:128 * m + rows, :, c]
    smT[:, 64:128, :, 2, :] = 0.0

    # hat tables per frame, packed flat with per-(slot,bx) shapes
    X, Y = np.meshgrid(np.arange(Nx, dtype=f64), np.arange(Ny, dtype=f64),
                       indexing="ij")
    ex_all = {}
    ey_all = {}
    for t in range(Nt):
        slot_t = FR_FULL if t == Nt - 1 else t % FR_FULL
        px = np.clip(X + flow[:, :, 0, t].astype(f64), 0.0, Nx - 1.0)
        py = np.clip(Y + flow[:, :, 1, t].astype(f64), 0.0, Ny - 1.0)
        pxt = px.reshape(NTX, BX, NTY, BY)   # [bx, vl, by, ul]
        pyt = py.reshape(NTX, BX, NTY, BY)
        ex_bx = []
        ey_bx = []
        for bx in range(NTX):
            WL = int(WLb[slot_t, bx])
            JLc = int(JLb[slot_t, bx])
            pxq = pxt[bx].transpose(0, 2, 1).reshape(128, NTY)  # q=4vl+ul
            pyq = pyt[bx].transpose(0, 2, 1).reshape(128, NTY)
            xrow = sx[slot_t, bx].astype(f64)[None, :, None] + \
                np.arange(WL, dtype=f64)[None, None, :]
            ex_bx.append(np.maximum(
                0.0, 1.0 - np.abs(xrow - pxq[:, :, None])
            ).astype(bf16))                                   # [128,NTY,WL]
            yrow = sy[slot_t, bx].astype(f64)[None, :, None] + \
                np.arange(JLc, dtype=f64)[None, None, :]
            eyv = np.maximum(0.0, 1.0 - np.abs(yrow - pyq[:, :, None]))
            ey_bx.append(eyv.astype(bf16).transpose(0, 2, 1))  # [128,JL,NTY]
        ex_all[t] = ex_bx
        ey_all[t] = ey_bx

    zz = np.zeros((1, 512), bf16)

    in_maps = []
    for r in range(NCORES):
        jl = jobs_for_core(r)
        kmX = np.zeros((NJOB, 128, KMW), bf16)
        for ji, (c, t) in enumerate(jl):
            kmr = (ksr[:, :, c] * maskf[:, :, c, t]).T   # [y, x]
            kmi = (ksi[:, :, c] * maskf[:, :, c, t]).T
            kmX[ji] = _km_pack(kmr, kmi).astype(bf16)
        smXc = np.zeros((Nc + C24, 128, 2, 3, Ny), f64)
        smXc[0:Nc] = smT
        for c in range(C24):
            smXc[Nc + c] = smT[C24 * r + c]
        slot_frames = [FR_FULL * r, FR_FULL * r + 1, FR_FULL * r + 2, Nt - 1]
        exd = np.zeros(g["ex_tot"], bf16)
        eyd = np.zeros(g["ey_tot"], bf16)
        for s in range(NSLOT):
            t = slot_frames[s]
            for bx in range(NTX):
                o = int(g["ex_off"][s, bx])
                exd[o:o + ex_all[t][bx].size] = ex_all[t][bx].ravel()
                o = int(g["ey_off"][s, bx])
                eyd[o:o + ey_all[t][bx].size] = ey_all[t][bx].ravel()
        in_maps.append({
            "kmX": kmX,
            "smX": smXc.astype(bf16),
            "ASd": ASd,
            "ASPMd": ASPMd,
            "exd": exd,
            "eyd": eyd,
            "zzd": zz,
        })
    return in_maps


def kernel(kspace_r, kspace_i, mask, smaps_r, smaps_i, flow):
    from concourse.bass_utils import run_bass_kernel_spmd

    flow = np.asarray(flow, np.float32)
    g = _geometry(flow)
    Dx = g["Dx"]

    key = (g["Dx"], g["Dy"], g["sx"].tobytes(), g["sy"].tobytes(),
           g["WLb"].tobytes(), g["JLb"].tobytes())
    if key not in _CACHE:
        _CACHE[key] = _build_program(g)
    nc = _CACHE[key]

    in_maps = _host_prep(
        np.asarray(kspace_r, np.float32), np.asarray(kspace_i, np.float32),
        np.asarray(mask, np.float32), np.asarray(smaps_r, np.float32),
        np.asarray(smaps_i, np.float32), flow, g)

    res = run_bass_kernel_spmd(nc, in_maps, core_ids=list(range(NCORES)))

    out = np.zeros((2, Nx, Ny), np.float64)
    for r in range(NCORES):
        o = res.results[r]["outp"].astype(np.float64)   # [NTX, 2, 128, 320]
        for bx in range(NTX):
            x0 = 32 * bx - Dx
            qlo = max(0, -x0)
            qhi = min(128, Nx - x0, 32 + 2 * Dx)
            for comp in range(2):
                out[comp, x0 + qlo:x0 + qhi, :] += o[bx, comp, qlo:qhi, :]
    return np.stack([out[0], out[1]], axis=-1).astype(np.float32)


# revision 40
# speedup vs baseline: 1.0453x; 1.0142x over previous
# Trainium2 Bass kernel for nn_BatchelorAdj (motion-compensated MRI recon
# adjoint):  out = sum_t W_t^T( sum_c conj(S_c) . IFFT2c(K_c . M_ct) )
#
# v4 design (cost-model driven):
#  - host precomputes km = kspace*mask per (coil,frame) in a stacked-chunk
#    bf16 layout so each IFFT matmul pass runs 5-instruction chains over a
#    640-long stacked contraction ([kmr;kmi] x [A_r;-A_i]); pass 1 merges
#    the 64-row x-tail of the r/i outputs into one 128-row chain, and pass
#    2 merges the two comps' x-tail chains via a stacked ASPM constant
#    (psum rows 0:64 = re tail, 64:128 = im tail) + partition-shift DMA.
#  - smaps: 2 planes (sr, si) resident in SBUF for all 16 coils (+2 f24
#    slots); the second coil-combine product pair uses a negative-stride
#    plane view of the image tile, so no duplicated sr plane is shipped.
#  - adjoint warp: host ships bf16 bilinear hat tables with per-(slot,bx)
#    widths and per-(slot,bx,ti) start offsets that are the union of the 8
#    cores' local windows (shared program => shared offsets), packed flat
#    in DRAM for full-rate DMA.
#  - schedule is slot-major: slot0 jobs; slot1 jobs || warp0; f24 jobs +
#    slot2 jobs || warp1+warp3; tail warp2. Warp table DMA spreads over
#    the whole fft timeline.
#  - per-bx psum bands [128, 324+2D] are flushed into per-bx bf16 SBUF
#    accumulators (Act copy on the first slot, DVE add after); host
#    applies x-shifts in the final cross-core reduce.
import math
import numpy as np

Nx = Ny = 320
Nc = 16
Nt = 25
NCORES = 8
BX, BY = 32, 4
NTX, NTY = Nx // BX, Ny // BY       # 10, 80
FR_FULL = 3
C24 = Nc // NCORES                  # 2
NSLOT = FR_FULL + 1                 # 4
NJOB = FR_FULL * Nc + C24           # 50 fft jobs per core
KMW = 2176                          # km tile width (see _km_pack)

_CACHE = {}


def _build_A():
    j = np.arange(Nx)
    F = np.exp(2j * np.pi * np.outer(j, j) / Nx) / np.sqrt(Nx)
    P = np.zeros((Nx, Nx))
    P[j, (j + Nx // 2) % Nx] = 1.0
    A = P @ F @ P
    return A.real.astype(np.float64), A.imag.astype(np.float64)


def _stack5(M, N):
    """Chunk the 640-row stack [M;N] of two 320-row mats into [5,128,cols]:
    (M0, N0, M1, N1, [M2;N2])."""
    out = np.zeros((5, 128, M.shape[1]), np.float64)
    out[0] = M[0:128]
    out[1] = N[0:128]
    out[2] = M[128:256]
    out[3] = N[128:256]
    out[4, 0:64] = M[256:320]
    out[4, 64:128] = N[256:320]
    return out


def _km_pack(kmr, kmi):
    """km [y,x] pair -> [128, 2176] layout.
    cols [0:1600]:   km5 chunks (kmr0,kmi0,kmr1,kmi1,[kmr2;kmi2]) full x.
    cols [1600:1664]: [kmi2 ; -kmr2] @ x 256:320 (partition-stacked)
    cols [1664:1920]: (kmi-y0, -kmr-y0, kmi-y1, -kmr-y1) @ x 256:320
    cols [1920:2176]: (kmr-y0, kmi-y0, kmr-y1, kmi-y1) @ x 256:320."""
    out = np.zeros((128, KMW), np.float64)
    km5 = _stack5(kmr, kmi)
    out[:, 0:1600] = km5.transpose(1, 0, 2).reshape(128, 1600)
    out[0:64, 1600:1664] = kmi[256:320, 256:320]
    out[64:128, 1600:1664] = -kmr[256:320, 256:320]
    out[:, 1664:1728] = kmi[0:128, 256:320]
    out[:, 1728:1792] = -kmr[0:128, 256:320]
    out[:, 1792:1856] = kmi[128:256, 256:320]
    out[:, 1856:1920] = -kmr[128:256, 256:320]
    out[:, 1920:1984] = kmr[0:128, 256:320]
    out[:, 1984:2048] = kmi[0:128, 256:320]
    out[:, 2048:2112] = kmr[128:256, 256:320]
    out[:, 2112:2176] = kmi[128:256, 256:320]
    return out


def _geometry(flow):
    """Union-local warp windows shared by all 8 cores.

    Returns dict with per-(slot,bx,ti) y/x window starts (sy, sx), per-
    (slot,bx) widths (JLb, WLb), flat ex/ey table offsets, and Dx/Dy."""
    f64 = np.float64
    Dx = int(math.ceil(np.abs(flow[:, :, 0, :]).max()))
    Dy = int(math.ceil(np.abs(flow[:, :, 1, :]).max()))
    W = 32 + 2 * Dx
    Xg, Yg = np.meshgrid(np.arange(Nx, dtype=f64), np.arange(Ny, dtype=f64),
                         indexing="ij")
    fminx = np.zeros((Nt, NTX, NTY), np.int64)
    fmaxx = np.zeros((Nt, NTX, NTY), np.int64)
    fminy = np.zeros((Nt, NTX, NTY), np.int64)
    fmaxy = np.zeros((Nt, NTX, NTY), np.int64)
    for t in range(Nt):
        px = np.clip(Xg + flow[:, :, 0, t].astype(f64), 0.0, Nx - 1.0)
        py = np.clip(Yg + flow[:, :, 1, t].astype(f64), 0.0, Ny - 1.0)
        fx = np.floor(px).astype(np.int64).reshape(NTX, BX, NTY, BY)
        fy = np.floor(py).astype(np.int64).reshape(NTX, BX, NTY, BY)
        fminx[t] = fx.min(axis=(1, 3))
        fmaxx[t] = fx.max(axis=(1, 3))
        fminy[t] = fy.min(axis=(1, 3))
        fmaxy[t] = fy.max(axis=(1, 3))

    sx = np.zeros((NSLOT, NTX, NTY), np.int64)
    ex_hi = np.zeros((NSLOT, NTX, NTY), np.int64)
    sy = np.zeros((NSLOT, NTX, NTY), np.int64)
    ey_hi = np.zeros((NSLOT, NTX, NTY), np.int64)
    for s in range(FR_FULL):
        frames = [FR_FULL * r + s for r in range(NCORES)]
        sx[s] = fminx[frames].min(axis=0)
        ex_hi[s] = fmaxx[frames].max(axis=0)
        sy[s] = fminy[frames].min(axis=0)
        ey_hi[s] = fmaxy[frames].max(axis=0)
    sx[FR_FULL], ex_hi[FR_FULL] = fminx[Nt - 1], fmaxx[Nt - 1]
    sy[FR_FULL], ey_hi[FR_FULL] = fminy[Nt - 1], fmaxy[Nt - 1]

    # x windows: PE psum writes need base partition in {0,32,64}, so pin
    # the window start at the band start and trim only the upper edge.
    bxlo = (32 * np.arange(NTX) - Dx)[None, :, None]
    sx = np.broadcast_to(bxlo, sx.shape).copy()
    WLb = np.minimum((ex_hi - sx + 2).max(axis=2), W)   # [NSLOT, NTX]
    JLb = (ey_hi - sy + 2).max(axis=2)
    sy = np.minimum(sy, Ny + Dy + 4 - JLb[:, :, None])

    ex_off = np.zeros((NSLOT, NTX), np.int64)
    ey_off = np.zeros((NSLOT, NTX), np.int64)
    pos = 0
    for s in range(NSLOT):
        for bx in range(NTX):
            ex_off[s, bx] = pos
            pos += 128 * NTY * int(WLb[s, bx])
    ex_tot = pos
    pos = 0
    for s in range(NSLOT):
        for bx in range(NTX):
            ey_off[s, bx] = pos
            pos += 128 * int(JLb[s, bx]) * NTY
    ey_tot = pos
    return dict(Dx=Dx, Dy=Dy, W=W, sx=sx, sy=sy, WLb=WLb, JLb=JLb,
                ex_off=ex_off, ey_off=ey_off, ex_tot=ex_tot, ey_tot=ey_tot)


def _build_program(g):
    from concourse import bass, bacc, tile, mybir

    Dx, Dy, W = g["Dx"], g["Dy"], g["W"]
    PWW = Ny + 2 * Dy + 4              # psum band columns (y + Dy offset)
    oy = g["sy"] + Dy                  # psum band col starts
    ox = g["sx"] - (32 * np.arange(NTX) - Dx)[None, :, None]  # band rows
    WLb, JLb = g["WLb"], g["JLb"]
    f32 = mybir.dt.float32
    bf16 = mybir.dt.bfloat16
    MULT = mybir.AluOpType.mult
    ADD = mybir.AluOpType.add
    SUB = mybir.AluOpType.subtract
    ACTF = mybir.ActivationFunctionType

    nc = bacc.Bacc("TRN2", target_bir_lowering=False, debug=False,
                   num_devices=NCORES)

    kmX = nc.dram_tensor("kmX", [NJOB, 128, KMW], bf16, kind="ExternalInput")
    smX = nc.dram_tensor("smX", [Nc + C24, 128, 2, 3, Ny], bf16,
                         kind="ExternalInput")
    ASd = nc.dram_tensor("ASd", [2, 5, 128, Ny], bf16, kind="ExternalInput")
    ASPMd = nc.dram_tensor("ASPMd", [5, 128, 128], bf16,
                           kind="ExternalInput")
    exd = nc.dram_tensor("exd", [g["ex_tot"]], bf16, kind="ExternalInput")
    eyd = nc.dram_tensor("eyd", [g["ey_tot"]], bf16, kind="ExternalInput")
    zzd = nc.dram_tensor("zzd", [1, 512], bf16, kind="ExternalInput")
    outp = nc.dram_tensor("outp", [NTX, 2, 128, Ny], bf16,
                          kind="ExternalOutput")

    from contextlib import ExitStack
    with tile.TileContext(nc) as tc, ExitStack() as ctx:
        const_pool = ctx.enter_context(tc.tile_pool(name="const", bufs=1))
        acc_pool = ctx.enter_context(tc.tile_pool(name="acc", bufs=1))
        aux_pool = ctx.enter_context(tc.tile_pool(name="aux", bufs=1))
        sm_pool = ctx.enter_context(tc.tile_pool(name="sm", bufs=1))
        sm24_pool = ctx.enter_context(tc.tile_pool(name="sm24", bufs=2))
        km_pool = ctx.enter_context(tc.tile_pool(name="km", bufs=2))
        t1_pool = ctx.enter_context(tc.tile_pool(name="t1", bufs=2))
        im_pool = ctx.enter_context(tc.tile_pool(name="im", bufs=2))
        scr_pool = ctx.enter_context(tc.tile_pool(name="scr", bufs=2))
        p_pool = ctx.enter_context(tc.tile_pool(name="prod", bufs=2))
        mid_pool = ctx.enter_context(tc.tile_pool(name="mid", bufs=2))
        imc_pool = ctx.enter_context(tc.tile_pool(name="imc", bufs=2))
        ex_pool = ctx.enter_context(tc.tile_pool(name="ex", bufs=2))
        ey_pool = ctx.enter_context(tc.tile_pool(name="ey", bufs=2))
        eyim_pool = ctx.enter_context(tc.tile_pool(name="eyim", bufs=2))
        psum_p1 = ctx.enter_context(tc.tile_pool(name="ps1", bufs=2,
                                                 space="PSUM"))
        psum_p2 = ctx.enter_context(tc.tile_pool(name="ps2", bufs=2,
                                                 space="PSUM"))
        psum_pw = ctx.enter_context(tc.tile_pool(name="psw", bufs=2,
                                                 space="PSUM"))

        # ---- job-0 km prefetch ahead of the constants ----
        km0 = km_pool.tile([128, KMW], bf16, tag="km", name="km0pre")
        nc.sync.dma_start(km0[:, :], kmX.ap()[0])

        # ---- constants ----
        AS = []
        for s in range(2):
            t = const_pool.tile([128, 5, Ny], bf16, tag=f"AS{s}")
            nc.sync.dma_start(t[:, :, :], ASd.ap()[s].transpose([1, 0, 2]))
            AS.append(t)
        ASPM = const_pool.tile([128, 5, 128], bf16, tag="ASPM")
        nc.sync.dma_start(ASPM[:, :, :], ASPMd.ap().transpose([1, 0, 2]))
        zz = const_pool.tile([1, 512], bf16, tag="zz")
        nc.sync.dma_start(zz[:, :], zzd.ap()[:, :])

        acc = {}
        for bx in range(NTX):
            for comp in range(2):
                t = acc_pool.tile([128, Ny], bf16, tag=f"acc{bx}_{comp}",
                                  name=f"acc{bx}_{comp}")
                acc[(bx, comp)] = t

        # ---- one FFT + coil-combine job ----
        def fft_job(job_id, sm_idx, auxt, first):
            if job_id == 0:
                km = km0
            else:
                km = km_pool.tile([128, KMW], bf16, tag="km",
                                  name=f"km{job_id}")
                nc.sync.dma_start(km[:, :], kmX.ap()[job_id])
            smt = get_sm(sm_idx)
            km5 = km[:, 0:1600].rearrange("p (c x) -> p c x", c=5, x=Ny)

            # pass 1: 4 plain chains + merged x-tail chain -> T1S [128,5,320]
            t1s = t1_pool.tile([128, 5, Ny], bf16, tag="t1s",
                               name=f"t1s{job_id}")
            for (c5, mlo, aset) in ((0, 0, 0), (1, 0, 1),
                                    (2, 128, 0), (3, 128, 1)):
                ps = psum_p1.tile([128, Ny], f32, tag="p1", name="p1")
                for c in range(5):
                    nc.tensor.matmul(ps[:, :], km5[:, c, mlo:mlo + 128],
                                     AS[aset][:, c, :],
                                     start=(c == 0), stop=(c == 4))
                nc.scalar.activation(t1s[:, c5, :], ps[:, :], ACTF.Copy)
            ps = psum_p1.tile([128, Ny], f32, tag="p1", name="p1m2")
            m2lhs = [
                km[:, 1920:2048],
                km[:, 1664:1792],
                km[:, 2048:2176],
                km[:, 1792:1920],
                km[:, 1536:1664],
            ]
            for c in range(5):
                nc.tensor.matmul(ps[:, :], m2lhs[c], AS[0][:, c, :],
                                 start=(c == 0), stop=(c == 4))
            nc.scalar.activation(t1s[:, 4, :], ps[:, :], ACTF.Copy)

            # pass 2: 4 full chains + merged x-tail chain -> im [128,2,3,320]
            imt = im_pool.tile([128, 2, 3, Ny], bf16, tag="im",
                               name=f"im{job_id}")
            for comp in range(2):
                for m in range(2):
                    ps = psum_p2.tile([128, Ny], f32, tag="p2", name="p2")
                    for c in range(5):
                        nc.tensor.matmul(
                            ps[:, :], AS[comp][:, c, 128 * m:128 * m + 128],
                            t1s[:, c, :], start=(c == 0), stop=(c == 4))
                    nc.scalar.activation(imt[:, comp, m, :], ps[:, :],
                                         ACTF.Copy)
            # merged tail: psum rows 0:64 = re x-tail, 64:128 = im x-tail
            ps = psum_p2.tile([128, Ny], f32, tag="p2", name="p2tail")
            for c in range(5):
                nc.tensor.matmul(ps[:, :], ASPM[:, c, :], t1s[:, c, :],
                                 start=(c == 0), stop=(c == 4))
            scr = scr_pool.tile([128, Ny], bf16, tag="scr",
                                name=f"scr{job_id}")
            nc.scalar.activation(scr[:, :], ps[:, :], ACTF.Copy)
            # rows 64:128 of the m2 planes are masked by zeroed smaps but
            # must hold finite values - cover them from scr too.
            nc.sync.dma_start(imt[:, 0, 2, :], scr[:, :])
            nc.sync.dma_start(imt[0:64, 1, 2, :], scr[64:128, :])
            nc.sync.dma_start(imt[64:128, 1, 2, :], scr[64:128, :])

            # coil combine: P = (sr*imr, si*imi, sr*imi, si*imr); real-part
            # accumulation on DVE, imag-part on Pool (engine balance)
            P = p_pool.tile([128, 4, 3, Ny], bf16, tag="P", name=f"P{job_id}")
            nc.vector.tensor_tensor(P[:, 0:2, :, :], smt[:, 0:2, :, :],
                                    imt[:, 0:2, :, :], MULT)
            nc.vector.tensor_tensor(P[:, 2:4, :, :], smt[:, 0:2, :, :],
                                    imt[:, 1::-1, :, :], MULT)
            if first:
                nc.vector.tensor_tensor(auxt[:, 0, :, :], P[:, 0, :, :],
                                        P[:, 1, :, :], ADD)
                nc.gpsimd.tensor_tensor(auxt[:, 1, :, :], P[:, 2, :, :],
                                        P[:, 3, :, :], SUB)
            else:
                nc.vector.tensor_tensor(auxt[:, 0, :, :], auxt[:, 0, :, :],
                                        P[:, 0, :, :], ADD)
                nc.vector.tensor_tensor(auxt[:, 0, :, :], auxt[:, 0, :, :],
                                        P[:, 1, :, :], ADD)
                nc.gpsimd.tensor_tensor(auxt[:, 1, :, :], auxt[:, 1, :, :],
                                        P[:, 2, :, :], ADD)
                nc.gpsimd.tensor_tensor(auxt[:, 1, :, :], auxt[:, 1, :, :],
                                        P[:, 3, :, :], SUB)

        sm_tiles = {}

        def get_sm(idx):
            if idx not in sm_tiles:
                pool = sm_pool if idx < Nc else sm24_pool
                t = pool.tile([128, 2, 3, Ny], bf16,
                              tag=("sm24" if idx >= Nc else f"sm{idx}"),
                              name=f"sm{idx}")
                nc.sync.dma_start(t[:, :, :, :], smX.ap()[idx])
                sm_tiles[idx] = t
            return sm_tiles[idx]

        # ---- warp chunks for one slot ----
        def warp_chunks(slot, auxt, first_acc):
            imc = imc_pool.tile([128, 2, NTX * NTY], bf16, tag="imc",
                                name=f"imc{slot}")

            def prep():
                for comp in range(2):
                    mid = mid_pool.tile([128, 3, 4, NTY], bf16, tag="mid",
                                        name=f"mid{slot}_{comp}")
                    for k in range(3):
                        nc.scalar.activation(
                            mid[:, k, :, :],
                            auxt[:, comp, k, :].rearrange(
                                "p (g ul) -> p g ul", g=NTY, ul=4)
                            .transpose([0, 2, 1]),
                            ACTF.Copy)
                    with nc.allow_non_contiguous_dma(reason="imc gather"):
                        for a in range(NTX):
                            k, a4 = a // 4, a % 4
                            nc.sync.dma_start(
                                imc[:, comp, NTY * a:NTY * a + NTY],
                                mid[32 * a4:32 * a4 + 32, k, :, :]
                                .rearrange("p ul g -> p (ul g)"))

            def mk_bx(bx):
                WL = int(WLb[slot, bx])
                JLc = int(JLb[slot, bx])

                def chunk():
                    ext = ex_pool.tile([128, NTY, WL], bf16, tag="ex",
                                       name=f"ex{slot}_{bx}")
                    nc.sync.dma_start(
                        ext[:, :, :],
                        exd.ap()[int(g["ex_off"][slot, bx]):
                                 int(g["ex_off"][slot, bx]) +
                                 128 * NTY * WL]
                        .rearrange("(p t w) -> p t w", p=128, t=NTY, w=WL))
                    eyt = ey_pool.tile([128, JLc, NTY], bf16, tag="ey",
                                       name=f"ey{slot}_{bx}")
                    nc.sync.dma_start(
                        eyt[:, :, :],
                        eyd.ap()[int(g["ey_off"][slot, bx]):
                                 int(g["ey_off"][slot, bx]) +
                                 128 * JLc * NTY]
                        .rearrange("(p j t) -> p j t", p=128, j=JLc, t=NTY))
                    eyim = eyim_pool.tile([128, 2, JLc, NTY], bf16,
                                          tag="eyim",
                                          name=f"eyim{slot}_{bx}")
                    nc.vector.tensor_tensor(
                        eyim[:, :, :, :],
                        eyt.unsqueeze(1).broadcast_to([128, 2, JLc, NTY]),
                        imc[:, :, NTY * bx:NTY * bx + NTY].unsqueeze(2)
                        .broadcast_to([128, 2, JLc, NTY]),
                        MULT)
                    pw = []
                    for comp in range(2):
                        p = psum_pw.tile([128, PWW], f32, tag=f"pw{comp}",
                                         name=f"pw{slot}_{bx}_{comp}")
                        nc.tensor.matmul(p[:, :], zz[0:1, 0:128],
                                         zz[0:1, 0:PWW], start=True,
                                         stop=False, skip_group_check=True)
                        pw.append(p)
                    for ti in range(NTY):
                        obr = int(ox[slot, bx, ti])
                        obc = int(oy[slot, bx, ti])
                        for comp in range(2):
                            nc.tensor.matmul(
                                pw[comp][obr:obr + WL, obc:obc + JLc],
                                ext[:, ti, :], eyim[:, comp, :, ti],
                                start=False, stop=(ti == NTY - 1),
                                skip_group_check=True)
                    for comp in range(2):
                        if first_acc:
                            nc.scalar.activation(
                                acc[(bx, comp)][:, :],
                                pw[comp][:, Dy:Dy + Ny], ACTF.Copy)
                        else:
                            nc.vector.tensor_tensor(
                                acc[(bx, comp)][:, :], acc[(bx, comp)][:, :],
                                pw[comp][:, Dy:Dy + Ny], ADD)
                return chunk

            return [prep] + [mk_bx(bx) for bx in range(NTX)]

        # ---- schedule (slot-major) ----
        aux = {}
        for slot in range(NSLOT):
            aux[slot] = aux_pool.tile([128, 2, 3, Ny], bf16,
                                      tag=f"aux{slot}", name=f"aux{slot}")

        def run_phase(jobs, pend):
            nonlocal job
            pi = 0
            for j, (smi, sl, first) in enumerate(jobs):
                fft_job(job, smi, aux[sl], first)
                job += 1
                goal = ((j + 1) * len(pend) + len(jobs) - 1) // len(jobs)
                while pi < min(goal, len(pend)):
                    pend[pi]()
                    pi += 1
            while pi < len(pend):
                pend[pi]()
                pi += 1

        job = 0
        # phase A: slot 0 + f24 jobs, no warps yet
        run_phase([(c, 0, c == 0) for c in range(Nc)] +
                  [(Nc + c, 3, c == 0) for c in range(C24)], [])
        # warp3 is ready after phase A; split its chunks across B and C
        w3 = warp_chunks(3, aux[3], first_acc=False)
        # phase B: slot 1 || warp0 + first part of warp3
        run_phase([(c, 1, c == 0) for c in range(Nc)],
                  warp_chunks(0, aux[0], first_acc=True) + w3[:6])
        # phase C: slot 2 || rest of warp3 (ready immediately) + warp1
        w1 = warp_chunks(1, aux[1], first_acc=False)
        run_phase([(c, 2, c == 0) for c in range(Nc)],
                  w3[6:] + [w1[0]] + w1[1:])
        assert job == NJOB
        for ch in warp_chunks(2, aux[2], first_acc=False):  # tail warp
            ch()

        for bx in range(NTX):
            for comp in range(2):
                nc.sync.dma_start(outp.ap()[bx, comp],
                                  acc[(bx, comp)][:, :])

    nc.compile()
    return nc


def _host_prep(kspace_r, kspace_i, mask, smaps_r, smaps_i, flow, g):
    import ml_dtypes
    bf16 = ml_dtypes.bfloat16
    f64 = np.float64
    Dx, Dy = g["Dx"], g["Dy"]
    sx, sy, WLb, JLb = g["sx"], g["sy"], g["WLb"], g["JLb"]

    Ar, Ai = _build_A()
    ASr = _stack5(Ar, -Ai)
    ASi = _stack5(Ai, Ar)
    ASd = np.stack([ASr, ASi]).astype(bf16)
    ASPMd = np.zeros((5, 128, 128), f64)
    ASPMd[:, :, 0:64] = ASr[:, :, 256:320]
    ASPMd[:, :, 64:128] = ASi[:, :, 256:320]
    ASPMd = ASPMd.astype(bf16)

    ksr = kspace_r.astype(f64)
    ksi = kspace_i.astype(f64)
    maskf = mask.astype(f64)

    def jobs_for_core(r):
        fr = [FR_FULL * r + s for s in range(FR_FULL)]
        out = [(c, fr[0]) for c in range(Nc)]
        out += [(C24 * r + c, Nt - 1) for c in range(C24)]
        out += [(c, fr[1]) for c in range(Nc)]
        out += [(c, fr[2]) for c in range(Nc)]
        return out

    # smaps [128, 2 planes (sr, si), 3 m, 320]; m2 rows 64:128 zeroed
    smT = np.zeros((Nc, 128, 2, 3, Ny), f64)
    for c in range(Nc):
        for m in range(3):
            rows = min(128, Nx - 128 * m)
            smT[c, 0:rows, 0, m, :] = smaps_r[128 * m:128 * m + rows, :, c]
            smT[c, 0:rows, 1, m, :] = smaps_i[128 * m:128 * m + rows, :, c]
    smT[:, 64:128, :, 2, :] = 0.0

    # hat tables per frame, packed flat with per-(slot,bx) shapes
    X, Y = np.meshgrid(np.arange(Nx, dtype=f64), np.arange(Ny, dtype=f64),
                       indexing="ij")
    ex_all = {}
    ey_all = {}
    for t in range(Nt):
        slot_t = FR_FULL if t == Nt - 1 else t % FR_FULL
        px = np.clip(X + flow[:, :, 0, t].astype(f64), 0.0, Nx - 1.0)
        py = np.clip(Y + flow[:, :, 1, t].astype(f64), 0.0, Ny - 1.0)
        pxt = px.reshape(NTX, BX, NTY, BY)   # [bx, vl, by, ul]
        pyt = py.reshape(NTX, BX, NTY, BY)
        ex_bx = []
        ey_bx = []
        for bx in range(NTX):
            WL = int(WLb[slot_t, bx])
            JLc = int(JLb[slot_t, bx])
            pxq = pxt[bx].transpose(0, 2, 1).reshape(128, NTY)  # q=4vl+ul
            pyq = pyt[bx].transpose(0, 2, 1).reshape(128, NTY)
            xrow = sx[slot_t, bx].astype(f64)[None, :, None] + \
                np.arange(WL, dtype=f64)[None, None, :]
            ex_bx.append(np.maximum(
                0.0, 1.0 - np.abs(xrow - pxq[:, :, None])
            ).astype(bf16))                                   # [128,NTY,WL]
            yrow = sy[slot_t, bx].astype(f64)[None, :, None] + \
                np.arange(JLc, dtype=f64)[None, None, :]
            eyv = np.maximum(0.0, 1.0 - np.abs(yrow - pyq[:, :, None]))
            ey_bx.append(eyv.astype(bf16).transpose(0, 2, 1))  # [128,JL,NTY]
        ex_all[t] = ex_bx
        ey_all[t] = ey_bx

    zz = np.zeros((1, 512), bf16)

    in_maps = []
    for r in range(NCORES):
        jl = jobs_for_core(r)
        kmX = np.zeros((NJOB, 128, KMW), bf16)
        for ji, (c, t) in enumerate(jl):
            kmr = (ksr[:, :, c] * maskf[:, :, c, t]).T   # [y, x]
            kmi = (ksi[:, :, c] * maskf[:, :, c, t]).T
            kmX[ji] = _km_pack(kmr, kmi).astype(bf16)
        smXc = np.zeros((Nc + C24, 128, 2, 3, Ny), f64)
        smXc[0:Nc] = smT
        for c in range(C24):
            smXc[Nc + c] = smT[C24 * r + c]
        slot_frames = [FR_FULL * r, FR_FULL * r + 1, FR_FULL * r + 2, Nt - 1]
        exd = np.zeros(g["ex_tot"], bf16)
        eyd = np.zeros(g["ey_tot"], bf16)
        for s in range(NSLOT):
            t = slot_frames[s]
            for bx in range(NTX):
                o = int(g["ex_off"][s, bx])
                exd[o:o + ex_all[t][bx].size] = ex_all[t][bx].ravel()
                o = int(g["ey_off"][s, bx])
                eyd[o:o + ey_all[t][bx].size] = ey_all[t][bx].ravel()
        in_maps.append({
            "kmX": kmX,
            "smX": smXc.astype(bf16),
            "ASd": ASd,
            "ASPMd": ASPMd,
            "exd": exd,
            "eyd": eyd,
            "zzd": zz,
        })
    return in_maps


def kernel(kspace_r, kspace_i, mask, smaps_r, smaps_i, flow):
    from concourse.bass_utils import run_bass_kernel_spmd

    flow = np.asarray(flow, np.float32)
    g = _geometry(flow)
    Dx = g["Dx"]

    key = (g["Dx"], g["Dy"], g["sx"].tobytes(), g["sy"].tobytes(),
           g["WLb"].tobytes(), g["JLb"].tobytes())
    if key not in _CACHE:
        _CACHE[key] = _build_program(g)
    nc = _CACHE[key]

    in_maps = _host_prep(
        np.asarray(kspace_r, np.float32), np.asarray(kspace_i, np.float32),
        np.asarray(mask, np.float32), np.asarray(smaps_r, np.float32),
        np.asarray(smaps_i, np.float32), flow, g)

    res = run_bass_kernel_spmd(nc, in_maps, core_ids=list(range(NCORES)))

    out = np.zeros((2, Nx, Ny), np.float64)
    for r in range(NCORES):
        o = res.results[r]["outp"].astype(np.float64)   # [NTX, 2, 128, 320]
        for bx in range(NTX):
            x0 = 32 * bx - Dx
            qlo = max(0, -x0)
            qhi = min(128, Nx - x0, 32 + 2 * Dx)
            for comp in range(2):
                out[comp, x0 + qlo:x0 + qhi, :] += o[bx, comp, qlo:qhi, :]
    return np.stack([out[0], out[1]], axis=-1).astype(np.float32)
